# revision 1
# baseline (speedup 1.0000x reference)
"""Trainium2 Bass kernel for nn_MemoryEfficientS6Compressor.

Key insight: the reference returns LN(W_out @ mean(ys[-8:]) + b_out) where
ys[-8:] are the last 8 positions of the LAST chunk (chunk-local t=24..31).
Chunks are independent, so only chunk 3 matters, and within it only:
  - xi (W_in proj) for chunk-local positions 14..31  (18 positions)
  - conv+silu (xc) for positions 17..31              (15 positions)
  - dt / gate / window-softmax for positions 24..31  (8 positions)
This cuts ~225 GFLOP to ~24 GFLOP.

Sharding: 7 conv groups (351 channels) -> cores 0..6; core 7 runs the same
SPMD program on zeroed weights. Cross-core sums (x_proj partials, W_out
partials) via AllReduce. All weights are pre-transposed on the host so the
device only does natural-layout loads; matmuls run in float32r (full-rate
fp32 PE mode).
"""

import os

import numpy as np

import concourse.bass as bass
import concourse.mybir as mybir
from concourse import bacc
import concourse.bass_utils as _BU
from concourse.bass_utils import run_bass_kernel_spmd

if os.environ.get("K_LDWOPT", "1") == "1" and not hasattr(_BU, "_k_ldw_patch"):
    _BU._k_ldw_patch = _BU.run_command

    def _run_command_ldw(argv, **kwargs):
        argv = ["--enable-ldw-opt=true" if a == "--enable-ldw-opt=false"
                else a for a in argv]
        return _BU._k_ldw_patch(argv, **kwargs)

    _BU.run_command = _run_command_ldw
from concourse.tile import TileContext

F32 = mybir.dt.float32
F32R = (mybir.dt.float32r if os.environ.get("K_F32R", "1") == "1"
        else mybir.dt.float32)
AF = mybir.ActivationFunctionType
ALU = mybir.AluOpType

SEQ, BATCH, D_MODEL = 128, 64, 2048
D_INNER, GROUPS, D_CONV = 2457, 7, 4
DT_RANK, WIN = 32, 8
GC = D_INNER // GROUPS          # 351 channels per group
NPOS = 18                        # xi positions (chunk-local 14..31)
NCONV = 15                       # conv output positions (17..31)
NT = 8                           # output positions (24..31)
TOK = NPOS * BATCH               # 1152
TOKC = NCONV * BATCH             # 960
TOKZ = NT * BATCH                # 512
# channel chunks (partition tiles) within the 351-channel group
CH = [(0, 128), (128, 128), (256, 95)]
NK = D_MODEL // 128              # 16 k-chunks over d_model

_cache = {}


class _StageDone(Exception):
    pass


def _r(ap):
    return ap.bitcast(F32R)


def _build(stage="F"):
    nc = bacc.Bacc("TRN2", target_bir_lowering=False, debug=False,
                   num_devices=8)

    xT = nc.dram_tensor("xT", [D_MODEL, TOK], F32R, kind="ExternalInput").ap()
    wig = nc.dram_tensor("wig", [D_MODEL, 2 * GC], F32R, kind="ExternalInput").ap()
    wc = nc.dram_tensor("wc", [GC, D_CONV * GC], F32R, kind="ExternalInput").ap()
    wo = nc.dram_tensor("wo", [GC + 1, D_MODEL], F32R, kind="ExternalInput").ap()
    wx = nc.dram_tensor("wx", [GC, DT_RANK], F32R, kind="ExternalInput").ap()
    wdt = nc.dram_tensor("wdt", [DT_RANK, GC], F32R, kind="ExternalInput").ap()
    biasv = nc.dram_tensor("biasv", [GC, 4], F32, kind="ExternalInput").ap()
    bxp = nc.dram_tensor("bxp", [DT_RANK, 1], F32, kind="ExternalInput").ap()
    lnwb = nc.dram_tensor("lnwb", [2, D_MODEL], F32, kind="ExternalInput").ap()
    cbias = nc.dram_tensor("cbias", [128, 8], F32, kind="ExternalInput").ap()
    out = nc.dram_tensor("out", [BATCH, D_MODEL], F32, kind="ExternalOutput").ap()

    with TileContext(nc) as tc:
        with (
            tc.tile_pool(name="xt", bufs=1) as xt_pool,
            tc.tile_pool(name="wig", bufs=4) as wig_pool,
            tc.tile_pool(name="wgt", bufs=1) as wgt_pool,
            tc.tile_pool(name="wo", bufs=2) as wo_pool,
            tc.tile_pool(name="act", bufs=1) as act_pool,
            tc.tile_pool(name="ek", bufs=4) as ek_pool,
            tc.tile_pool(name="sc", bufs=1) as sc_pool,
            tc.tile_pool(name="ps", bufs=1, space="PSUM") as ps_pool,
            tc.tile_pool(name="dram", bufs=1, space="DRAM") as dram_pool,
        ):
            def _phases():
                # ---- small persistent loads ---------------------------------
                bias_sb = []
                for m, (c0, cw) in enumerate(CH):
                    b = sc_pool.tile([cw, 4], F32, tag=f"bias{m}", name=f"bias{m}")
                    nc.sync.dma_start(out=b[:], in_=biasv[c0:c0 + cw, :])
                    bias_sb.append(b)
                cb_sb = sc_pool.tile([128, 8], F32, tag="cb", name="cb")
                nc.sync.dma_start(out=cb_sb[:], in_=cbias[:, :])
                bxp_sb = sc_pool.tile([DT_RANK, 1], F32, tag="bxp", name="bxp")
                nc.sync.dma_start(out=bxp_sb[:], in_=bxp[:, :])
                wdt_sb = sc_pool.tile([DT_RANK, GC], F32R, tag="wdt", name="wdt")
                nc.sync.dma_start(out=wdt_sb[:], in_=wdt[:, :])
                wx_sb = []
                for m, (c0, cw) in enumerate(CH):
                    t = sc_pool.tile([cw, DT_RANK], F32R, tag=f"wx{m}", name=f"wx{m}")
                    nc.sync.dma_start(out=t[:], in_=wx[c0:c0 + cw, :])
                    wx_sb.append(t)
                wc_sb = []
                for m, (c0, cw) in enumerate(CH):
                    t = wgt_pool.tile([cw, D_CONV * GC], F32R, tag=f"wc{m}", name=f"wc{m}")
                    nc.sync.dma_start(out=t[:], in_=wc[c0:c0 + cw, :])
                    wc_sb.append(t)

                # ---- phase A: xi = W_in @ x (+b_in), z = sigmoid(W_gate@x+b_g) --
                xt_sb = [xt_pool.tile([128, TOK], F32R, tag=f"xt{k}", name=f"xt{k}")
                         for k in range(NK)]
                xi_sb = []
                for m, (c0, cw) in enumerate(CH):
                    pxi = [ps_pool.tile([cw, 384], F32, tag=f"pxi{n}", name=f"pxi{n}")
                           for n in range(3)]
                    for k in range(NK):
                        if m == 0:
                            nc.sync.dma_start(out=xt_sb[k][:],
                                              in_=xT[k * 128:(k + 1) * 128, :])
                        wg = wig_pool.tile([128, cw], F32R, tag="wig", name="wig")
                        nc.sync.dma_start(
                            out=wg[:],
                            in_=wig[k * 128:(k + 1) * 128,
                                    2 * c0:2 * c0 + cw])
                        st, sp = (k == 0), (k == NK - 1)
                        for n in range(3):
                            nc.tensor.matmul(
                                pxi[n][:], wg[:],
                                xt_sb[k][:, n * 384:(n + 1) * 384],
                                start=st, stop=sp)
                    xi = act_pool.tile([cw, TOK], F32R, tag=f"xi{m}", name=f"xi{m}")
                    for n in range(3):
                        nc.scalar.activation(xi[:, n * 384:(n + 1) * 384],
                                             pxi[n][:], AF.Identity,
                                             bias=bias_sb[m][:, 0:1])
                    xi_sb.append(xi)

                if stage == "A":
                    nc.gpsimd.dma_start(out=out[0:64, 0:TOK], in_=xi_sb[0][0:64, :])
                    return
                # ---- phase B: grouped conv (as 4-tap matmul) + silu -------------
                # conv output tokens: n=1 -> tokens 448..960 (positions 10..17,
                # needed by xp) computed FIRST so the xp AllReduce can launch
                # early; n=0 -> tokens 0..448.
                xc_sb = [act_pool.tile([cw, TOKC], F32, tag=f"xc{m}", name=f"xc{m}")
                         for m, (c0, cw) in enumerate(CH)]
                conv_chunks = [(448, 512), (0, 448)]
                for t0, tw in conv_chunks:
                    for m, (c0, cw) in enumerate(CH):
                        pc = ps_pool.tile([cw, tw], F32, tag="pc", bufs=3, name="pconv")
                        for kc, (k0, kw) in enumerate(CH):
                            for j in range(D_CONV):
                                nc.tensor.matmul(
                                    pc[:],
                                    wc_sb[kc][:, j * GC + c0:j * GC + c0 + cw],
                                    xi_sb[kc][:, t0 + j * BATCH:
                                               t0 + j * BATCH + tw],
                                    start=(kc == 0 and j == 0),
                                    stop=(kc == 2 and j == D_CONV - 1))
                        sgt = ek_pool.tile([cw, tw], F32, tag="ek", name="sgt")
                        nc.scalar.activation(sgt[:], pc[:], AF.Sigmoid,
                                             bias=bias_sb[m][:, 1:2])
                        nc.vector.scalar_tensor_tensor(
                            xc_sb[m][:, t0:t0 + tw], pc[:], bias_sb[m][:, 1:2],
                            sgt[:], op0=ALU.add, op1=ALU.mult)
                    if t0 == 448:
                        # ---- phase C: xp partial + AllReduce --------------------
                        xcr = []
                        for kc, (k0, kw) in enumerate(CH):
                            t = act_pool.tile([kw, TOKZ], F32R, tag=f"xcr{kc}",
                                              name=f"xcr{kc}")
                            nc.scalar.copy(t[:], xc_sb[kc][:, 448:960])
                            xcr.append(t)
                        pxp = ps_pool.tile([DT_RANK, TOKZ], F32, tag="pc", bufs=3, name="pxp")
                        for kc, (k0, kw) in enumerate(CH):
                            nc.tensor.matmul(pxp[:], wx_sb[kc][:], xcr[kc][:],
                                             start=(kc == 0), stop=(kc == 2))
                        xp_sb = sc_pool.tile([DT_RANK, TOKZ], F32, tag="xp", name="xp")
                        nc.scalar.activation(xp_sb[:], pxp[:], AF.Identity,
                                             bias=bxp_sb[:, 0:1])
                        xp_part = dram_pool.tile([DT_RANK, TOKZ], F32, name="xp_part")
                        xp_red = dram_pool.tile([DT_RANK, TOKZ], F32, name="xp_red")
                        nc.sync.dma_start(out=xp_part[:], in_=xp_sb[:])
                        nc.gpsimd.collective_compute(
                            "AllReduce", ALU.add,
                            replica_groups=[list(range(8))],
                            ins=[xp_part.opt()], outs=[xp_red.opt()])
                        xps = sc_pool.tile([DT_RANK, TOKZ], F32R, tag="xps", name="xps")
                        nc.gpsimd.dma_start(out=xps[:], in_=xp_red[:])

                # ---- phase Z: gate z = sigmoid(W_gate@x + b_g) ------------------
                # emitted after the xp AllReduce launch so PE fills the
                # collective's latency with useful work
                sigz_sb = []
                for m, (c0, cw) in enumerate(CH):
                    pz = ps_pool.tile([cw, TOKZ], F32, tag="pz", name="pz")
                    for k in range(NK):
                        wgz = wig_pool.tile([128, cw], F32R, tag="wig",
                                            name="wigz")
                        nc.sync.dma_start(
                            out=wgz[:],
                            in_=wig[k * 128:(k + 1) * 128,
                                    2 * c0 + cw:2 * c0 + 2 * cw])
                        nc.tensor.matmul(pz[:], wgz[:],
                                         xt_sb[k][:, TOK - TOKZ:],
                                         start=(k == 0), stop=(k == NK - 1))
                    sz = act_pool.tile([cw, TOKZ], F32, tag=f"sigz{m}",
                                       name=f"sigz{m}")
                    nc.scalar.activation(sz[:], pz[:], AF.Sigmoid,
                                         bias=bias_sb[m][:, 2:3])
                    sigz_sb.append(sz)

                if stage == "B":
                    nc.gpsimd.dma_start(out=out[0:64, 0:TOKC], in_=xc_sb[0][0:64, :])
                    return
                if stage == "C":
                    nc.gpsimd.dma_start(out=out[0:32, 0:TOKZ], in_=xps[:])
                    return
                # ---- phase D: dt chain + windowed softmax attention -------------
                cext = [sc_pool.tile([cw, BATCH], F32R,
                                     tag=f"cext{m}", name=f"cext{m}")
                        for m, (c0, cw) in enumerate(CH)]
                ones1f = sc_pool.tile([1, BATCH], F32, tag="ones1f", name="ones1f")
                nc.vector.memset(ones1f[:], 1.0)
                ones1 = sc_pool.tile([1, BATCH], F32R, tag="ones1", name="ones1")
                nc.scalar.copy(ones1[:], ones1f[:])
                for m, (c0, cw) in enumerate(CH):
                    pdt = ps_pool.tile([cw, TOKZ], F32, tag="pdt")
                    nc.tensor.matmul(pdt[:], wdt_sb[:, c0:c0 + cw],
                                     xps[:], start=True, stop=True)
                    usp = ek_pool.tile([cw, TOKZ], F32, tag="ek", name="usp")
                    nc.scalar.activation(usp[:], pdt[:], AF.Exp)
                    dt = act_pool.tile([cw, TOKZ], F32, tag=f"xi{m}", name=f"dt{m}")
                    nc.scalar.activation(dt[:], usp[:], AF.Ln, bias=1.0)
                    # E_k = exp(k*(dt+1e-4)); S = sum_k E_k (k=0..7);
                    # num = sum_k E_k * xc[:, k*64 : k*64+512]
                    S = act_pool.tile([cw, TOKZ], F32, tag=f"S{m}", name=f"Ssum{m}")
                    num = act_pool.tile([cw, TOKZ], F32, tag=f"num{m}", name=f"num{m}")
                    tmp = act_pool.tile([cw, TOKZ], F32, tag=f"tmp{m}", name=f"tmp{m}")
                    ek_prev = None
                    for k in range(1, WIN):
                        ek = ek_pool.tile([cw, TOKZ], F32, tag="ek", name="ek")
                        nc.scalar.activation(ek[:], dt[:], AF.Exp,
                                             scale=float(k),
                                             bias=cb_sb[0:cw, k - 1:k])
                        xck = xc_sb[m][:, k * BATCH:k * BATCH + TOKZ]
                        if k == 1:
                            nc.vector.tensor_mul(num[:], ek[:], xck)
                            ek_prev = ek
                        elif k == 2:
                            nc.vector.scalar_tensor_tensor(
                                S[:], ek[:], 1.0, ek_prev[:],
                                op0=ALU.add, op1=ALU.add)
                            nc.vector.tensor_mul(tmp[:], ek[:], xck)
                            nc.vector.tensor_add(num[:], num[:], tmp[:])
                        else:
                            nc.vector.tensor_add(S[:], S[:], ek[:])
                            nc.vector.tensor_mul(tmp[:], ek[:], xck)
                            nc.vector.tensor_add(num[:], num[:], tmp[:])
                    nc.vector.tensor_add(num[:], num[:], xc_sb[m][:, 0:TOKZ])
                    sinv = ek_pool.tile([cw, TOKZ], F32, tag="ek", name="sinv")
                    scr = ek_pool.tile([cw, TOKZ], F32, tag="ek", name="scr")
                    nc.vector.reciprocal_approx_accurate(out=sinv[:], in_=S[:],
                                                         scratch=scr[:])
                    nc.vector.tensor_mul(num[:], num[:], sinv[:])
                    # ys = (num + D*xc[t]) * sigz ; then sum over the 8 t's
                    nc.vector.scalar_tensor_tensor(
                        tmp[:], xc_sb[m][:, 7 * BATCH:7 * BATCH + TOKZ],
                        bias_sb[m][:, 3:4], num[:], op0=ALU.mult, op1=ALU.add)
                    nc.vector.tensor_mul(tmp[:], tmp[:], sigz_sb[m][:])
                    nc.vector.tensor_add(S[:, 0:256], tmp[:, 0:256], tmp[:, 256:512])
                    nc.vector.tensor_add(S[:, 0:128], S[:, 0:128], S[:, 128:256])
                    nc.vector.tensor_add(cext[m][:], S[:, 0:64], S[:, 64:128])

                if stage == "D":
                    for m, (c0, cw) in enumerate(CH):
                        nc.gpsimd.dma_start(out=out[0:cw, m * 64:(m + 1) * 64],
                                            in_=cext[m][:])
                    return
                # ---- phase E: out partial = cext @ woT (+b_out row), AllReduce --
                po = [ps_pool.tile([BATCH, 512], F32,
                                   tag=(f"pxi{n}" if n < 3 else "pz"),
                                   name=f"po{n}")
                      for n in range(4)]
                wo_rows = [(0, 128), (128, 128), (256, 95), (351, 1)]
                for kc, (r0, rw) in enumerate(wo_rows):
                    wot = wo_pool.tile([rw, D_MODEL], F32R, tag="wo", name="wo")
                    nc.sync.dma_start(out=wot[:], in_=wo[r0:r0 + rw, :])
                    lhs = cext[kc][:] if kc < 3 else ones1[:]
                    for n in range(4):
                        nc.tensor.matmul(po[n][:], lhs,
                                         wot[:, n * 512:(n + 1) * 512],
                                         start=(kc == 0), stop=(kc == 3))
                outp = sc_pool.tile([BATCH, D_MODEL], F32, tag="outp", name="outp")
                for n in range(4):
                    nc.scalar.activation(outp[:, n * 512:(n + 1) * 512],
                                         po[n][:], AF.Copy)
                op_part = dram_pool.tile([BATCH, D_MODEL], F32, name="op_part")
                op_red = dram_pool.tile([BATCH, D_MODEL], F32, name="op_red")
                nc.sync.dma_start(out=op_part[:], in_=outp[:])
                nc.gpsimd.collective_compute(
                    "AllReduce", ALU.add, replica_groups=[list(range(8))],
                    ins=[op_part.opt()], outs=[op_red.opt()])
                osb = sc_pool.tile([BATCH, D_MODEL], F32, tag="osb", name="osb")
                nc.sync.dma_start(out=osb[:], in_=op_red[:])

                if stage == "E":
                    nc.sync.dma_start(out=out[:], in_=osb[:])
                    return
                # ---- phase F: layernorm over d_model (free dim) -----------------
                mu = sc_pool.tile([BATCH, 1], F32, tag="mu", name="mu")
                nc.vector.reduce_sum(mu[:], osb[:], axis=mybir.AxisListType.X)
                mus = sc_pool.tile([BATCH, 1], F32, tag="mus", name="mus")
                nc.scalar.mul(mus[:], mu[:], 1.0 / D_MODEL)
                cen = sc_pool.tile([BATCH, D_MODEL], F32, tag="cen", name="cen")
                nc.vector.tensor_scalar_sub(cen[:], osb[:], mus[:])
                sq = sc_pool.tile([BATCH, D_MODEL], F32, tag="osb", name="sq")
                vs = sc_pool.tile([BATCH, 1], F32, tag="vs", name="vs")
                nc.scalar.activation(sq[:], cen[:], AF.Square,
                                     accum_out=vs[:])
                std = sc_pool.tile([BATCH, 1], F32, tag="std", name="stdt")
                nc.scalar.activation(std[:], vs[:], AF.Sqrt,
                                     scale=1.0 / D_MODEL,
                                     bias=cb_sb[0:BATCH, 7:8])
                rstd = sc_pool.tile([BATCH, 1], F32, tag="rstd", name="rstd")
                nc.vector.reciprocal(rstd[:], std[:])
                lnw_sb = xt_pool.tile([1, D_MODEL], F32R, tag="xt0", name="lnw1")
                nc.gpsimd.dma_start(out=lnw_sb[:], in_=lnwb[0:1, :])
                lnb_sb = xt_pool.tile([1, D_MODEL], F32R, tag="xt1", name="lnb1")
                nc.gpsimd.dma_start(out=lnb_sb[:], in_=lnwb[1:2, :])
                for n in range(4):
                    pw = ps_pool.tile([BATCH, 512], F32,
                                      tag=(f"pxi{n}" if n < 3 else "pz"),
                                      name="pw")
                    pb = ps_pool.tile([BATCH, 512], F32, tag="pc", bufs=3,
                                      name="pb")
                    nc.tensor.matmul(pw[:], ones1[:],
                                     lnw_sb[:, n * 512:(n + 1) * 512],
                                     start=True, stop=True)
                    nc.tensor.matmul(pb[:], ones1[:],
                                     lnb_sb[:, n * 512:(n + 1) * 512],
                                     start=True, stop=True)
                    nc.vector.scalar_tensor_tensor(
                        cen[:, n * 512:(n + 1) * 512],
                        cen[:, n * 512:(n + 1) * 512], rstd[:], pw[:],
                        op0=ALU.mult, op1=ALU.mult)
                    nc.vector.tensor_add(cen[:, n * 512:(n + 1) * 512],
                                         cen[:, n * 512:(n + 1) * 512],
                                         pb[:])
                nc.sync.dma_start(out=out[:], in_=cen[:])

            _phases()
    nc.compile()
    return nc


def _host_prep(inputs):
    f = lambda k: np.ascontiguousarray(np.asarray(inputs[k], dtype=np.float32))
    x, W_in, b_in = f("x"), f("W_in"), f("b_in")
    W_gate, b_gate = f("W_gate"), f("b_gate")
    W_conv, b_conv = f("W_conv"), f("b_conv")
    W_xproj, b_xproj = f("W_xproj"), f("b_xproj")
    W_dt, Dparam = f("W_dt"), f("Dparam")
    W_out, b_out = f("W_out"), f("b_out")
    ln_w, ln_b = f("ln_w"), f("ln_b")

    xT = np.ascontiguousarray(
        x[SEQ - NPOS:].reshape(TOK, D_MODEL).T)          # [2048, 1152]
    lnwb = np.ascontiguousarray(np.stack([ln_w, ln_b]))  # [2, 2048]
    cbias = np.zeros((128, 8), np.float32)
    for k in range(1, WIN):
        cbias[:, k - 1] = float(k) * 1e-4
    cbias[:, 7] = 1e-5

    in_maps = []
    for g in range(8):
        if g < GROUPS:
            ch = slice(GC * g, GC * (g + 1))
            WinT = W_in[ch].T                            # [2048, 351]
            WgT = W_gate[ch].T
            wig = np.empty((D_MODEL, 2 * GC), np.float32)
            off = 0
            for c0, cw in CH:
                wig[:, off:off + cw] = WinT[:, c0:c0 + cw]
                wig[:, off + cw:off + 2 * cw] = WgT[:, c0:c0 + cw]
                off += 2 * cw
            wcm = np.ascontiguousarray(
                W_conv[ch].transpose(1, 2, 0).reshape(GC, D_CONV * GC))
            wom = np.zeros((GC + 1, D_MODEL), np.float32)
            wom[:GC] = W_out[:, ch].T / float(WIN)
            if g == 0:
                wom[GC] = b_out
            wxm = np.ascontiguousarray(W_xproj[:DT_RANK, ch].T)
            wdtm = np.ascontiguousarray(W_dt[ch].T)
            biasm = np.ascontiguousarray(
                np.stack([b_in[ch], b_conv[ch], b_gate[ch], Dparam[ch]], 1))
            bxpm = (b_xproj[:DT_RANK] if g == 0
                    else np.zeros(DT_RANK, np.float32)).reshape(DT_RANK, 1)
            bxpm = np.ascontiguousarray(bxpm)
        else:
            wig = np.zeros((D_MODEL, 2 * GC), np.float32)
            wcm = np.zeros((GC, D_CONV * GC), np.float32)
            wom = np.zeros((GC + 1, D_MODEL), np.float32)
            wxm = np.zeros((GC, DT_RANK), np.float32)
            wdtm = np.zeros((DT_RANK, GC), np.float32)
            biasm = np.zeros((GC, 4), np.float32)
            bxpm = np.zeros((DT_RANK, 1), np.float32)
        in_maps.append({
            "xT": xT, "wig": np.ascontiguousarray(wig), "wc": wcm,
            "wo": wom, "wx": wxm, "wdt": wdtm, "biasv": biasm,
            "bxp": bxpm, "lnwb": lnwb, "cbias": cbias,
        })
    return in_maps


def kernel(**inputs):
    if "nc" not in _cache:
        import os
        _cache["nc"] = _build(os.environ.get("K_STAGE", "F"))
    in_maps = _host_prep(inputs)
    res = run_bass_kernel_spmd(_cache["nc"], in_maps, list(range(8)))
    return res.results[0]["out"]



# revision 13
# speedup vs baseline: 1.1470x; 1.1470x over previous
"""Trainium2 Bass kernel for nn_MemoryEfficientS6Compressor.

Key insight: the reference returns LN(W_out @ mean(ys[-8:]) + b_out) where
ys[-8:] are the last 8 positions of the LAST chunk (chunk-local t=24..31).
Chunks are independent, so only chunk 3 matters, and within it only:
  - xi (W_in proj) for chunk-local positions 14..31  (18 positions)
  - conv+silu (xc) for positions 17..31              (15 positions)
  - dt / gate / window-softmax for positions 24..31  (8 positions)

Sharding: 7 conv groups (351 channels) -> cores 0..6; core 7 runs the same
SPMD program on zeroed weights. Cross-core sums (x_proj partials, W_out
partials) via AllReduce.

Perf notes (vs the f32r baseline at 274us):
  - fp16 weights/activations for all matmuls (PE 1 cyc/row, half DMA and
    LDWEIGHTS), bf16 only for exp-range tiles (E_k, S, num).
  - wig ([d_model, xi|z] packed) loaded ONCE and reused by both the W_in
    and W_gate passes (baseline streamed it twice = 96 DMAs).
  - DMA issue split across sync/vector/gpsimd queues; x/wig first.
  - E_k = exp(k*dt) built from 2 table exps (k=1,3) + products on DVE /
    Pool; avoids act-table thrash (exp->ln->exp per chunk in baseline).
  - windowed-softmax arithmetic split across DVE + Pool + Act engines,
    pipelined per channel-chunk with the remaining conv matmuls.
  - output partial AllReduce in fp16 (half payload), Shared output tensor.
"""

import os

import numpy as np

import concourse.bass as bass
import concourse.mybir as mybir
from concourse import bacc
import concourse.bass_utils as _BU
from concourse.bass_utils import run_bass_kernel_spmd

if os.environ.get("K_LDWOPT", "1") == "1" and not hasattr(_BU, "_k_ldw_patch"):
    _BU._k_ldw_patch = _BU.run_command

    def _run_command_ldw(argv, **kwargs):
        argv = ["--enable-ldw-opt=true" if a == "--enable-ldw-opt=false"
                else a for a in argv]
        return _BU._k_ldw_patch(argv, **kwargs)

    _BU.run_command = _run_command_ldw
from concourse.tile import TileContext

F32 = mybir.dt.float32
F16 = mybir.dt.float16
BF16 = mybir.dt.bfloat16
AF = mybir.ActivationFunctionType
ALU = mybir.AluOpType

SEQ, BATCH, D_MODEL = 128, 64, 2048
D_INNER, GROUPS, D_CONV = 2457, 7, 4
DT_RANK, WIN = 32, 8
GC = D_INNER // GROUPS          # 351 channels per group
NPOS = 18                        # xi positions (chunk-local 14..31)
NCONV = 15                       # conv output positions (17..31)
NT = 8                           # output positions (24..31)
TOK = NPOS * BATCH               # 1152
TOKC = NCONV * BATCH             # 960
TOKZ = NT * BATCH                # 512
# channel chunks (partition tiles) within the 351-channel group
CH = [(0, 128), (128, 128), (256, 95)]
NK = D_MODEL // 128              # 16 k-chunks over d_model
AR16 = os.environ.get("K_AR16", "1") == "1"

_cache = {}


def _build():
    nc = bacc.Bacc("TRN2", target_bir_lowering=False, debug=False,
                   num_devices=8)

    xT = nc.dram_tensor("xT", [D_MODEL, TOK], F16, kind="ExternalInput").ap()
    wig = nc.dram_tensor("wig", [D_MODEL, 2 * GC], F16, kind="ExternalInput").ap()
    wc = nc.dram_tensor("wc", [GC, D_CONV * GC], F16, kind="ExternalInput").ap()
    wo = nc.dram_tensor("wo", [GC + 1, D_MODEL], F16, kind="ExternalInput").ap()
    wx = nc.dram_tensor("wx", [GC, DT_RANK], F16, kind="ExternalInput").ap()
    wdt = nc.dram_tensor("wdt", [DT_RANK, GC], F16, kind="ExternalInput").ap()
    biasv = nc.dram_tensor("biasv", [GC, 4], F32, kind="ExternalInput").ap()
    bxp = nc.dram_tensor("bxp", [DT_RANK, 1], F32, kind="ExternalInput").ap()
    lnwb = nc.dram_tensor("lnwb", [2, D_MODEL], F16, kind="ExternalInput").ap()
    out = nc.dram_tensor("out", [BATCH, D_MODEL], F32, kind="ExternalOutput").ap()

    ODT = F16 if AR16 else F32
    xp_part = nc.dram_tensor("xp_part", [DT_RANK, TOKZ], F32,
                             kind="Internal").ap()
    xp_red = nc.dram_tensor("xp_red", [DT_RANK, TOKZ], F32,
                            kind="Internal", addr_space="Shared").ap()
    op_part = nc.dram_tensor("op_part", [BATCH, D_MODEL], ODT,
                             kind="Internal").ap()
    op_red = nc.dram_tensor("op_red", [BATCH, D_MODEL], ODT,
                            kind="Internal", addr_space="Shared").ap()

    with TileContext(nc) as tc:
        with (
            tc.tile_pool(name="xt", bufs=1) as xt_pool,
            tc.tile_pool(name="wig", bufs=1) as wig_pool,
            tc.tile_pool(name="wgt", bufs=1) as wgt_pool,
            tc.tile_pool(name="wo", bufs=1) as wo_pool,
            tc.tile_pool(name="act", bufs=1) as act_pool,
            tc.tile_pool(name="ek", bufs=1) as ek_pool,
            tc.tile_pool(name="tmp", bufs=3) as tmp_pool,
            tc.tile_pool(name="sc", bufs=1) as sc_pool,
            tc.tile_pool(name="ps", bufs=1, space="PSUM") as ps_pool,
        ):
            # ---- DMA issue: sync carries x/wig (phase-A critical path), ----
            # ---- vector carries small tiles, gpsimd carries wc/wo.       ----
            xt_sb = []
            wig_sb = []
            for k in range(NK):
                xt_t = xt_pool.tile([128, TOK], F16, tag=f"xt{k}", name=f"xt{k}")
                nc.sync.dma_start(out=xt_t[:], in_=xT[k * 128:(k + 1) * 128, :])
                xt_sb.append(xt_t)
                wg_t = wig_pool.tile([128, 2 * GC], F16, tag=f"wig{k}",
                                     name=f"wig{k}")
                nc.sync.dma_start(out=wg_t[:],
                                  in_=wig[k * 128:(k + 1) * 128, :])
                wig_sb.append(wg_t)

            bias_sb = []
            for m, (c0, cw) in enumerate(CH):
                b = sc_pool.tile([cw, 4], F32, tag=f"bias{m}", name=f"bias{m}")
                nc.scalar.dma_start(out=b[:], in_=biasv[c0:c0 + cw, :])
                bias_sb.append(b)
            wx_sb = []
            for m, (c0, cw) in enumerate(CH):
                t = sc_pool.tile([cw, DT_RANK], F16, tag=f"wx{m}", name=f"wx{m}")
                nc.scalar.dma_start(out=t[:], in_=wx[c0:c0 + cw, :])
                wx_sb.append(t)
            wdt_sb = sc_pool.tile([DT_RANK, GC], F16, tag="wdt", name="wdt")
            nc.scalar.dma_start(out=wdt_sb[:], in_=wdt[:, :])
            bxp_sb = sc_pool.tile([DT_RANK, 1], F32, tag="bxp", name="bxp")
            nc.scalar.dma_start(out=bxp_sb[:], in_=bxp[:, :])
            lnw_sb = sc_pool.tile([1, D_MODEL], F16, tag="lnw", name="lnw")
            nc.scalar.dma_start(out=lnw_sb[:], in_=lnwb[0:1, :])
            lnb_sb = sc_pool.tile([1, D_MODEL], F16, tag="lnb", name="lnb")
            nc.scalar.dma_start(out=lnb_sb[:], in_=lnwb[1:2, :])
            ones1 = sc_pool.tile([1, BATCH], F16, tag="ones1", name="ones1")
            nc.vector.memset(ones1[:], 1.0)
            cb = sc_pool.tile([128, 3], F32, tag="cb", name="cb")
            nc.vector.memset(cb[:, 0:1], 1e-4)
            nc.vector.memset(cb[:, 1:2], 3e-4)
            nc.vector.memset(cb[:, 2:3], 1e-5)

            wc_sb = []
            for m, (c0, cw) in enumerate(CH):
                t = wgt_pool.tile([cw, D_CONV * GC], F16, tag=f"wc{m}",
                                  name=f"wc{m}")
                nc.gpsimd.dma_start(out=t[:], in_=wc[c0:c0 + cw, :])
                wc_sb.append(t)
            wo_rows = [(0, 128), (128, 128), (256, 95), (351, 1)]
            wo_sb = []
            for r, (r0, rw) in enumerate(wo_rows):
                t = wo_pool.tile([rw, D_MODEL], F16, tag=f"wo{r}", name=f"wo{r}")
                nc.gpsimd.dma_start(out=t[:], in_=wo[r0:r0 + rw, :])
                wo_sb.append(t)

            # ---- phase A: xi = W_in @ x (+b_in), fp16 ------------------------
            # n-chunk outer so x/wig k-tiles are consumed in DMA-arrival order
            xi_sb = [act_pool.tile([cw, TOK], F16, tag=f"xi{m}", name=f"xi{m}")
                     for m, (c0, cw) in enumerate(CH)]
            for n in range(3):
                pxi = [ps_pool.tile([cw, 384], F32, tag=f"pxi{m}",
                                    name=f"pxi{m}_{n}")
                       for m, (c0, cw) in enumerate(CH)]
                for k in range(NK):
                    for m, (c0, cw) in enumerate(CH):
                        nc.tensor.matmul(
                            pxi[m][:],
                            wig_sb[k][:, 2 * c0:2 * c0 + cw],
                            xt_sb[k][:, n * 384:(n + 1) * 384],
                            start=(k == 0), stop=(k == NK - 1))
                for m, (c0, cw) in enumerate(CH):
                    nc.scalar.activation(xi_sb[m][:, n * 384:(n + 1) * 384],
                                         pxi[m][:], AF.Identity,
                                         bias=bias_sb[m][:, 0:1])

            # ---- conv chunk n=1 (tokens 448..960) for the xp path -----------
            xc_sb = [act_pool.tile([cw, TOKC], F16, tag=f"xc{m}", name=f"xc{m}")
                     for m, (c0, cw) in enumerate(CH)]
            for m, (c0, cw) in enumerate(CH):
                pc = ps_pool.tile([cw, 512], F32, tag="pc", bufs=3, name="pconv")
                for kc, (k0, kw) in enumerate(CH):
                    for j in range(D_CONV):
                        nc.tensor.matmul(
                            pc[:],
                            wc_sb[kc][:, j * GC + c0:j * GC + c0 + cw],
                            xi_sb[kc][:, 448 + j * BATCH:448 + j * BATCH + 512],
                            start=(kc == 0 and j == 0),
                            stop=(kc == 2 and j == D_CONV - 1))
                nc.scalar.activation(xc_sb[m][:, 448:960], pc[:], AF.Silu,
                                     bias=bias_sb[m][:, 1:2])

            # ---- xp partial + AllReduce -------------------------------------
            pxp = ps_pool.tile([DT_RANK, TOKZ], F32, tag="pc", bufs=3,
                               name="pxp")
            for kc, (k0, kw) in enumerate(CH):
                nc.tensor.matmul(pxp[:], wx_sb[kc][:],
                                 xc_sb[kc][:, 448:960],
                                 start=(kc == 0), stop=(kc == 2))
            xp_sb = sc_pool.tile([DT_RANK, TOKZ], F32, tag="xp", name="xp")
            nc.scalar.activation(xp_sb[:], pxp[:], AF.Identity,
                                 bias=bxp_sb[:, 0:1])
            nc.sync.dma_start(out=xp_part[:], in_=xp_sb[:])
            nc.gpsimd.collective_compute(
                "AllReduce", ALU.add, replica_groups=[list(range(8))],
                ins=[xp_part.opt()], outs=[xp_red.opt()])
            xps_f = sc_pool.tile([DT_RANK, TOKZ], F32, tag="xpsf", name="xpsf")
            nc.gpsimd.dma_start(out=xps_f[:], in_=xp_red[:])
            xps = sc_pool.tile([DT_RANK, TOKZ], F16, tag="xps", name="xps")
            nc.scalar.activation(xps[:], xps_f[:], AF.Copy)

            # ---- gate z = sigmoid(W_gate@x + b_g), fills the AR latency -----
            sigz_sb = []
            for m, (c0, cw) in enumerate(CH):
                pz = ps_pool.tile([cw, TOKZ], F32, tag="pz", name="pz")
                for k in range(NK):
                    nc.tensor.matmul(pz[:],
                                     wig_sb[k][:, 2 * c0 + cw:2 * c0 + 2 * cw],
                                     xt_sb[k][:, TOK - TOKZ:],
                                     start=(k == 0), stop=(k == NK - 1))
                sz = act_pool.tile([cw, TOKZ], F16, tag=f"sigz{m}",
                                   name=f"sigz{m}")
                nc.scalar.activation(sz[:], pz[:], AF.Sigmoid,
                                     bias=bias_sb[m][:, 2:3])
                sigz_sb.append(sz)

            # ---- dt-chain prologue: usp = exp(u), dt = ln(1+usp) for all m --
            # (batched so the act-table switches exp->ln->exp only once)
            usp_sb, dt_sb = [], []
            for m, (c0, cw) in enumerate(CH):
                pdt = ps_pool.tile([cw, TOKZ], F32, tag="pz" if m == 2 else "pc",
                                   bufs=1 if m == 2 else 3, name=f"pdt{m}")
                nc.tensor.matmul(pdt[:], wdt_sb[:, c0:c0 + cw],
                                 xps[:], start=True, stop=True)
                usp = act_pool.tile([cw, TOKZ], F16, tag=f"usp{m}",
                                    name=f"usp{m}")
                nc.scalar.activation(usp[:], pdt[:], AF.Exp)
                usp_sb.append(usp)
            for m, (c0, cw) in enumerate(CH):
                dt = act_pool.tile([cw, TOKZ], F32, tag=f"dt{m}", name=f"dt{m}")
                nc.scalar.activation(dt[:], usp_sb[m][:], AF.Ln, bias=1.0)
                dt_sb.append(dt)

            # ---- per m: conv chunk n=0 (PE) + windowed softmax (DVE/Pool) ---
            # E_k = exp(k*(dt+1e-4)); table exps only for k=1,3; the rest are
            # products:  E2=E1*E1, E6=E3*E3 (DVE)  E5=E2*E3, E7=E2*E5 (Pool).
            # S = 1 + sum_k E_k ; num = sum_k E_k * xc[:, k*64:k*64+512].
            cext = []
            S_sb = []
            for m, (c0, cw) in enumerate(CH):
                # conv n=0: tokens 0..448
                pc = ps_pool.tile([cw, 448], F32, tag="pc", bufs=3,
                                  name=f"pconv0_{m}")
                for kc, (k0, kw) in enumerate(CH):
                    for j in range(D_CONV):
                        nc.tensor.matmul(
                            pc[:],
                            wc_sb[kc][:, j * GC + c0:j * GC + c0 + cw],
                            xi_sb[kc][:, j * BATCH:j * BATCH + 448],
                            start=(kc == 0 and j == 0),
                            stop=(kc == 2 and j == D_CONV - 1))
                nc.scalar.activation(xc_sb[m][:, 0:448], pc[:], AF.Silu,
                                     bias=bias_sb[m][:, 1:2])

                dt = dt_sb[m]
                xck = lambda k: xc_sb[m][:, k * BATCH:k * BATCH + TOKZ]
                e = {}
                for ki, k in enumerate((1, 3)):
                    e[k] = ek_pool.tile([cw, TOKZ], BF16, tag=f"e{m}_{k}",
                                        name=f"e{m}_{k}")
                    nc.scalar.activation(e[k][:], dt[:], AF.Exp,
                                         scale=float(k),
                                         bias=cb[0:cw, ki:ki + 1])
                for k, tg in ((2, f"e{m}_2"), (6, f"e{m}_6"),
                              (4, f"e{m}_4"), (5, f"e{m}_5"), (7, f"e{m}_7")):
                    e[k] = ek_pool.tile([cw, TOKZ], BF16, tag=tg,
                                        name=f"e{m}_{k}")
                nc.vector.tensor_mul(e[2][:], e[1][:], e[1][:])
                nc.vector.tensor_mul(e[6][:], e[3][:], e[3][:])
                nc.gpsimd.tensor_mul(e[4][:], e[1][:], e[3][:])
                nc.gpsimd.tensor_mul(e[5][:], e[2][:], e[3][:])
                nc.gpsimd.tensor_mul(e[7][:], e[2][:], e[5][:])

                S = act_pool.tile([cw, TOKZ], BF16, tag=f"S{m}", name=f"S{m}")
                num = act_pool.tile([cw, TOKZ], BF16, tag=f"num{m}",
                                    name=f"num{m}")
                # S = 1 + E1  (then += E2..E7)
                nc.vector.tensor_scalar_add(S[:], e[1][:], 1.0)
                nc.vector.tensor_mul(num[:], e[1][:], xck(1))
                engs = [nc.vector, nc.gpsimd]
                for i, k in enumerate((2, 3, 5, 6, 7)):
                    engs[i % 2].tensor_add(S[:], S[:], e[k][:])
                nc.vector.tensor_add(S[:], S[:], e[4][:])
                for i, k in enumerate((2, 3, 4, 5, 6, 7)):
                    tmp = tmp_pool.tile([cw, TOKZ], BF16, tag="tmp",
                                        name=f"tmp{m}_{k}")
                    engs[i % 2].tensor_mul(tmp[:], e[k][:], xck(k))
                    engs[(i + 1) % 2].tensor_add(num[:], num[:], tmp[:])
                S_sb.append(S)
                cext.append((num, m))

            # reciprocals batched (one act-table switch), then finals
            sinv_sb = []
            for m, (c0, cw) in enumerate(CH):
                sf = tmp_pool.tile([cw, TOKZ], F32, tag="sf", bufs=2,
                                   name=f"sf{m}")
                nc.scalar.copy(sf[:], S_sb[m][:])
                sinv = tmp_pool.tile([cw, TOKZ], F32, tag=f"sinv{m}",
                                     name=f"sinv{m}")
                nc.vector.reciprocal_approx_fast(out=sinv[:], in_=sf[:])
                sinv_sb.append(sinv)
            cext_sb = []
            for m, (c0, cw) in enumerate(CH):
                num, _ = cext[m]
                xck = lambda k: xc_sb[m][:, k * BATCH:k * BATCH + TOKZ]
                loc = tmp_pool.tile([cw, TOKZ], F16, tag="tmp", name=f"loc{m}")
                # loc = num/S + xc0/S ... note k=0 term: num += xc0 first
                nc.vector.tensor_add(num[:], num[:], xck(0))
                nc.vector.tensor_mul(loc[:], num[:], sinv_sb[m][:])
                ys = tmp_pool.tile([cw, TOKZ], F16, tag="tmp", name=f"ys{m}")
                nc.vector.scalar_tensor_tensor(
                    ys[:], xck(7), bias_sb[m][:, 3:4], loc[:],
                    op0=ALU.mult, op1=ALU.add)
                nc.gpsimd.tensor_mul(ys[:], ys[:], sigz_sb[m][:])
                t1 = tmp_pool.tile([cw, 256], F16, tag="tmp", name=f"t1{m}")
                nc.vector.tensor_add(t1[:], ys[:, 0:256], ys[:, 256:512])
                nc.vector.tensor_add(t1[:, 0:128], t1[:, 0:128], t1[:, 128:256])
                ce = sc_pool.tile([cw, BATCH], F16, tag=f"cext{m}",
                                  name=f"cext{m}")
                nc.vector.tensor_add(ce[:], t1[:, 0:64], t1[:, 64:128])
                cext_sb.append(ce)

            # ---- out partial = cext @ woT (+b_out row), AllReduce -----------
            po = [ps_pool.tile([BATCH, 512], F32,
                               tag=(f"pxi{n}" if n < 3 else "pz"),
                               name=f"po{n}")
                  for n in range(4)]
            for kc in range(4):
                lhs = cext_sb[kc][:] if kc < 3 else ones1[:]
                for n in range(4):
                    nc.tensor.matmul(po[n][:], lhs,
                                     wo_sb[kc][:, n * 512:(n + 1) * 512],
                                     start=(kc == 0), stop=(kc == 3))
            outp = sc_pool.tile([BATCH, D_MODEL], ODT, tag="outp", name="outp")
            for n in range(4):
                nc.scalar.activation(outp[:, n * 512:(n + 1) * 512],
                                     po[n][:], AF.Copy)
            nc.sync.dma_start(out=op_part[:], in_=outp[:])
            nc.gpsimd.collective_compute(
                "AllReduce", ALU.add, replica_groups=[list(range(8))],
                ins=[op_part.opt()], outs=[op_red.opt()])
            osb = sc_pool.tile([BATCH, D_MODEL], ODT, tag="osb", name="osb")
            nc.gpsimd.dma_start(out=osb[:], in_=op_red[:])

            # ---- layernorm over d_model (free dim) --------------------------
            mu = sc_pool.tile([BATCH, 1], F32, tag="mu", name="mu")
            nc.vector.reduce_sum(mu[:], osb[:], axis=mybir.AxisListType.X)
            mus = sc_pool.tile([BATCH, 1], F32, tag="mus", name="mus")
            nc.scalar.mul(mus[:], mu[:], 1.0 / D_MODEL)
            cen = sc_pool.tile([BATCH, D_MODEL], F16, tag="cen", name="cen")
            nc.vector.tensor_scalar_sub(cen[:], osb[:], mus[:])
            sq = sc_pool.tile([BATCH, D_MODEL], F16, tag="outp", name="sq")
            vs = sc_pool.tile([BATCH, 1], F32, tag="vs", name="vs")
            nc.scalar.activation(sq[:], cen[:], AF.Square, accum_out=vs[:])
            std = sc_pool.tile([BATCH, 1], F32, tag="std", name="std")
            nc.scalar.activation(std[:], vs[:], AF.Sqrt,
                                 scale=1.0 / D_MODEL, bias=cb[0:BATCH, 2:3])
            rstd = sc_pool.tile([BATCH, 1], F32, tag="rstd", name="rstd")
            nc.vector.reciprocal(rstd[:], std[:])
            for n in range(4):
                pw = ps_pool.tile([BATCH, 512], F32,
                                  tag=(f"pxi{n}" if n < 3 else "pz"),
                                  name="pw")
                pb = ps_pool.tile([BATCH, 512], F32, tag="pc", bufs=3,
                                  name="pb")
                nc.tensor.matmul(pw[:], ones1[:],
                                 lnw_sb[:, n * 512:(n + 1) * 512],
                                 start=True, stop=True)
                nc.tensor.matmul(pb[:], ones1[:],
                                 lnb_sb[:, n * 512:(n + 1) * 512],
                                 start=True, stop=True)
                fin = sc_pool.tile([BATCH, 512], F32, tag="fin", bufs=2,
                                   name=f"fin{n}")
                nc.vector.scalar_tensor_tensor(
                    fin[:], cen[:, n * 512:(n + 1) * 512], rstd[:], pw[:],
                    op0=ALU.mult, op1=ALU.mult)
                nc.vector.tensor_add(fin[:], fin[:], pb[:])
                nc.sync.dma_start(out=out[:, n * 512:(n + 1) * 512],
                                  in_=fin[:])

    nc.compile()
    return nc


def _host_prep(inputs):
    f = lambda k: np.ascontiguousarray(np.asarray(inputs[k], dtype=np.float32))
    x, W_in, b_in = f("x"), f("W_in"), f("b_in")
    W_gate, b_gate = f("W_gate"), f("b_gate")
    W_conv, b_conv = f("W_conv"), f("b_conv")
    W_xproj, b_xproj = f("W_xproj"), f("b_xproj")
    W_dt, Dparam = f("W_dt"), f("Dparam")
    W_out, b_out = f("W_out"), f("b_out")
    ln_w, ln_b = f("ln_w"), f("ln_b")

    xT = np.ascontiguousarray(
        x[SEQ - NPOS:].reshape(TOK, D_MODEL).T).astype(np.float16)
    lnwb = np.ascontiguousarray(np.stack([ln_w, ln_b])).astype(np.float16)

    in_maps = []
    for g in range(8):
        if g < GROUPS:
            ch = slice(GC * g, GC * (g + 1))
            WinT = W_in[ch].T                            # [2048, 351]
            WgT = W_gate[ch].T
            wig = np.empty((D_MODEL, 2 * GC), np.float32)
            off = 0
            for c0, cw in CH:
                wig[:, off:off + cw] = WinT[:, c0:c0 + cw]
                wig[:, off + cw:off + 2 * cw] = WgT[:, c0:c0 + cw]
                off += 2 * cw
            wcm = np.ascontiguousarray(
                W_conv[ch].transpose(1, 2, 0).reshape(GC, D_CONV * GC))
            wom = np.zeros((GC + 1, D_MODEL), np.float32)
            wom[:GC] = W_out[:, ch].T / float(WIN)
            if g == 0:
                wom[GC] = b_out
            wxm = np.ascontiguousarray(W_xproj[:DT_RANK, ch].T)
            wdtm = np.ascontiguousarray(W_dt[ch].T)
            biasm = np.ascontiguousarray(
                np.stack([b_in[ch], b_conv[ch], b_gate[ch], Dparam[ch]], 1))
            bxpm = (b_xproj[:DT_RANK] if g == 0
                    else np.zeros(DT_RANK, np.float32)).reshape(DT_RANK, 1)
            bxpm = np.ascontiguousarray(bxpm)
        else:
            wig = np.zeros((D_MODEL, 2 * GC), np.float32)
            wcm = np.zeros((GC, D_CONV * GC), np.float32)
            wom = np.zeros((GC + 1, D_MODEL), np.float32)
            wxm = np.zeros((GC, DT_RANK), np.float32)
            wdtm = np.zeros((DT_RANK, GC), np.float32)
            biasm = np.zeros((GC, 4), np.float32)
            bxpm = np.zeros((DT_RANK, 1), np.float32)
        in_maps.append({
            "xT": xT,
            "wig": np.ascontiguousarray(wig).astype(np.float16),
            "wc": wcm.astype(np.float16),
            "wo": wom.astype(np.float16),
            "wx": wxm.astype(np.float16),
            "wdt": wdtm.astype(np.float16),
            "biasv": biasm, "bxp": bxpm, "lnwb": lnwb,
        })
    return in_maps


def kernel(**inputs):
    if "nc" not in _cache:
        _cache["nc"] = _build()
    in_maps = _host_prep(inputs)
    res = run_bass_kernel_spmd(_cache["nc"], in_maps, list(range(8)))
    return res.results[0]["out"]


# revision 15
# speedup vs baseline: 1.4403x; 1.2557x over previous
"""Trainium2 Bass kernel for nn_MemoryEfficientS6Compressor.

Key insight: the reference returns LN(W_out @ mean(ys[-8:]) + b_out) where
ys[-8:] are the last 8 positions of the LAST chunk (chunk-local t=24..31).
Chunks are independent, so only chunk 3 matters, and within it only:
  - xi (W_in proj) for chunk-local positions 14..31  (18 positions)
  - conv+silu (xc) for positions 17..31              (15 positions)
  - dt / gate / window-softmax for positions 24..31  (8 positions)

Sharding: 7 conv groups (351 channels) -> cores 0..6; core 7 runs the same
SPMD program on zeroed weights. Cross-core sums (x_proj partials, W_out
partials) via AllReduce.

Schedule (all fp16 matmuls; bf16 only for exp-range tiles):
  1. DMA the xp-critical slices first (x cols 448.., W_in block) so the
     x_proj AllReduce launches ~35us in instead of ~90us.
  2. A-passes over tokens 448..960, 960..1152 -> conv(448..960) -> xp ->
     AllReduce(fp16).  While it flies: A-pass 0..448, conv(0..448),
     dt-proj, gate.
  3. Windowed softmax on merged [128, 3*512] tiles (3 channel-chunks per
     op) split across Act (table exps) / DVE / Pool.
  4. out partial -> fp16 AllReduce (Shared output) -> layernorm.
"""

import os

import numpy as np

import concourse.bass as bass
import concourse.mybir as mybir
from concourse import bacc
from concourse.bass_utils import run_bass_kernel_spmd
from concourse.tile import TileContext

F32 = mybir.dt.float32
F16 = mybir.dt.float16
BF16 = mybir.dt.bfloat16
AF = mybir.ActivationFunctionType
ALU = mybir.AluOpType

SEQ, BATCH, D_MODEL = 128, 64, 2048
D_INNER, GROUPS, D_CONV = 2457, 7, 4
DT_RANK, WIN = 32, 8
GC = D_INNER // GROUPS          # 351 channels per group
NPOS = 18                        # xi positions (chunk-local 14..31)
NCONV = 15                       # conv output positions (17..31)
TOK = NPOS * BATCH               # 1152
TOKC = NCONV * BATCH             # 960
TOKZ = 8 * BATCH                 # 512
CH = [(0, 128), (128, 128), (256, 95)]
NK = D_MODEL // 128              # 16 k-chunks over d_model
HI = 448                         # token split: [448..1152] is xp-critical

_cache = {}


def _build():
    nc = bacc.Bacc("TRN2", target_bir_lowering=False, debug=False,
                   num_devices=8)

    xT = nc.dram_tensor("xT", [D_MODEL, TOK], F16, kind="ExternalInput").ap()
    wig = nc.dram_tensor("wig", [D_MODEL, 2 * GC], F16, kind="ExternalInput").ap()
    wc = nc.dram_tensor("wc", [GC, D_CONV * GC], F16, kind="ExternalInput").ap()
    wo = nc.dram_tensor("wo", [GC + 1, D_MODEL], F16, kind="ExternalInput").ap()
    wx = nc.dram_tensor("wx", [GC, DT_RANK], F16, kind="ExternalInput").ap()
    wdt = nc.dram_tensor("wdt", [DT_RANK, GC], F16, kind="ExternalInput").ap()
    biasv = nc.dram_tensor("biasv", [GC, 4], F32, kind="ExternalInput").ap()
    bxp = nc.dram_tensor("bxp", [DT_RANK, 1], F32, kind="ExternalInput").ap()
    lnwb = nc.dram_tensor("lnwb", [2, D_MODEL], F16, kind="ExternalInput").ap()
    out = nc.dram_tensor("out", [BATCH, D_MODEL], F32, kind="ExternalOutput").ap()

    xp_part = nc.dram_tensor("xp_part", [DT_RANK, TOKZ], F16,
                             kind="Internal").ap()
    xp_red = nc.dram_tensor("xp_red", [DT_RANK, TOKZ], F16,
                            kind="Internal", addr_space="Shared").ap()
    op_part = nc.dram_tensor("op_part", [BATCH, D_MODEL], F16,
                             kind="Internal").ap()
    op_red = nc.dram_tensor("op_red", [BATCH, D_MODEL], F16,
                            kind="Internal", addr_space="Shared").ap()

    with TileContext(nc) as tc:
        with (
            tc.tile_pool(name="xt", bufs=1) as xt_pool,
            tc.tile_pool(name="wig", bufs=1) as wig_pool,
            tc.tile_pool(name="wgt", bufs=1) as wgt_pool,
            tc.tile_pool(name="wo", bufs=1) as wo_pool,
            tc.tile_pool(name="act", bufs=1) as act_pool,
            tc.tile_pool(name="ek", bufs=1) as ek_pool,
            tc.tile_pool(name="tmp", bufs=3) as tmp_pool,
            tc.tile_pool(name="sc", bufs=1) as sc_pool,
            tc.tile_pool(name="ps", bufs=1, space="PSUM") as ps_pool,
        ):
            # ---- DMA: xp-critical first (x hi-cols + W_in), paired k ------
            # host packs wig = [W_in.T | W_gate.T] (each [2048, 351])
            xthi_sb, wxi_sb = [], []
            for p in range(NK // 2):
                th = xt_pool.tile([128, 2, TOK - HI], F16, tag=f"xth{p}",
                                  name=f"xth{p}")
                nc.sync.dma_start(
                    out=th[:, :, :],
                    in_=xT[256 * p:256 * (p + 1), HI:].rearrange(
                        "(two p) t -> p two t", two=2))
                xthi_sb.append(th)
                tw = wig_pool.tile([128, 2, GC], F16, tag=f"wxi{p}",
                                   name=f"wxi{p}")
                nc.sync.dma_start(
                    out=tw[:, :, :],
                    in_=wig[256 * p:256 * (p + 1), 0:GC].rearrange(
                        "(two p) t -> p two t", two=2))
                wxi_sb.append(tw)
            # deferred: x lo-cols + W_gate
            xtlo_sb, wz_sb = [], []
            for p in range(NK // 2):
                tl = xt_pool.tile([128, 2, HI], F16, tag=f"xtl{p}",
                                  name=f"xtl{p}")
                nc.sync.dma_start(
                    out=tl[:, :, :],
                    in_=xT[256 * p:256 * (p + 1), 0:HI].rearrange(
                        "(two p) t -> p two t", two=2))
                xtlo_sb.append(tl)
                tz = wig_pool.tile([128, 2, GC], F16, tag=f"wz{p}",
                                   name=f"wz{p}")
                nc.sync.dma_start(
                    out=tz[:, :, :],
                    in_=wig[256 * p:256 * (p + 1), GC:].rearrange(
                        "(two p) t -> p two t", two=2))
                wz_sb.append(tz)

            def xt_hi(k, t0, t1):       # tokens in [HI, 1152)
                return xthi_sb[k // 2][:, k % 2, t0 - HI:t1 - HI]

            def xt_lo(k, t0, t1):       # tokens in [0, HI)
                return xtlo_sb[k // 2][:, k % 2, t0:t1]

            def w_xi(k, c0, cw):
                return wxi_sb[k // 2][:, k % 2, c0:c0 + cw]

            def w_z(k, c0, cw):
                return wz_sb[k // 2][:, k % 2, c0:c0 + cw]

            # small tiles on the scalar queue
            bias_sb = []
            for m, (c0, cw) in enumerate(CH):
                b = sc_pool.tile([cw, 4], F32, tag=f"bias{m}", name=f"bias{m}")
                nc.scalar.dma_start(out=b[:], in_=biasv[c0:c0 + cw, :])
                bias_sb.append(b)
            wx_sb = []
            for m, (c0, cw) in enumerate(CH):
                t = sc_pool.tile([cw, DT_RANK], F16, tag=f"wx{m}", name=f"wx{m}")
                nc.scalar.dma_start(out=t[:], in_=wx[c0:c0 + cw, :])
                wx_sb.append(t)
            wdt_sb = sc_pool.tile([DT_RANK, GC], F16, tag="wdt", name="wdt")
            nc.scalar.dma_start(out=wdt_sb[:], in_=wdt[:, :])
            bxp_sb = sc_pool.tile([DT_RANK, 1], F32, tag="bxp", name="bxp")
            nc.scalar.dma_start(out=bxp_sb[:], in_=bxp[:, :])
            lnw_sb = sc_pool.tile([1, D_MODEL], F16, tag="lnw", name="lnw")
            nc.scalar.dma_start(out=lnw_sb[:], in_=lnwb[0:1, :])
            lnb_sb = sc_pool.tile([1, D_MODEL], F16, tag="lnb", name="lnb")
            nc.scalar.dma_start(out=lnb_sb[:], in_=lnwb[1:2, :])

            ones1 = sc_pool.tile([1, BATCH], F16, tag="ones1", name="ones1")
            nc.vector.memset(ones1[:], 1.0)
            cb = sc_pool.tile([128, 8], F32, tag="cb", name="cb")
            for k in range(1, 8):
                nc.vector.memset(cb[:, k - 1:k], float(k) * 1e-4)
            nc.vector.memset(cb[:, 7:8], 1e-5)

            # conv + out weights on the gpsimd queue
            wc_sb = []
            for m, (c0, cw) in enumerate(CH):
                t = wgt_pool.tile([cw, D_CONV * GC], F16, tag=f"wc{m}",
                                  name=f"wc{m}")
                nc.gpsimd.dma_start(out=t[:], in_=wc[c0:c0 + cw, :])
                wc_sb.append(t)
            wo_rows = [(0, 128), (128, 128), (256, 95), (351, 1)]
            wo_sb = []
            for r, (r0, rw) in enumerate(wo_rows):
                t = wo_pool.tile([rw, D_MODEL], F16, tag=f"wo{r}", name=f"wo{r}")
                nc.gpsimd.dma_start(out=t[:], in_=wo[r0:r0 + rw, :])
                wo_sb.append(t)

            # merged activation tiles: [128, 3*N] spanning channel-chunks
            xi_sb = [act_pool.tile([cw, TOK], F16, tag=f"xi{m}", name=f"xi{m}")
                     for m, (c0, cw) in enumerate(CH)]
            xc_all = act_pool.tile([128, 3 * TOKC], F16, tag="xc", name="xc")
            sigz_all = act_pool.tile([128, 3 * TOKZ], F16, tag="sigz",
                                     name="sigz")
            usp_all = act_pool.tile([128, 3 * TOKZ], F16, tag="usp", name="usp")
            nc.vector.memset(usp_all[64:128, 2 * TOKZ:3 * TOKZ], 0.0)
            nc.vector.memset(sigz_all[64:128, 2 * TOKZ:3 * TOKZ], 0.0)
            nc.vector.memset(xc_all[64:128, 2 * TOKC:3 * TOKC], 0.0)
            dt_all = act_pool.tile([128, 3 * TOKZ], F32, tag="dt", name="dt")
            EE = ek_pool.tile([128, 7 * 3 * TOKZ], BF16, tag="EE", name="EE")

            def xc3(m, t0, t1):
                return xc_all[0:CH[m][1], m * TOKC + t0:m * TOKC + t1]

            def xcv(k):     # [128, 3, 512] window view
                v = xc_all[:, :].rearrange("p (m t) -> p m t", m=3)
                return v[:, :, k * BATCH:k * BATCH + TOKZ]

            def eev(k):     # [128, 3, 512] view of plane k
                v = EE[:, (k - 1) * 3 * TOKZ:k * 3 * TOKZ]
                return v.rearrange("p (m t) -> p m t", m=3)

            def ee2(k):     # [128, 1536] view of plane k
                return EE[:, (k - 1) * 3 * TOKZ:k * 3 * TOKZ]

            # ---- A-passes over hi tokens, then conv(448..960), xp, AR ------
            def a_pass(t0, t1, xt_f):
                pxi = [ps_pool.tile([cw, t1 - t0], F32, tag=f"pxi{m}",
                                    name=f"pxi{m}_{t0}")
                       for m, (c0, cw) in enumerate(CH)]
                for k in range(NK):
                    for m, (c0, cw) in enumerate(CH):
                        nc.tensor.matmul(pxi[m][:], w_xi(k, c0, cw),
                                         xt_f(k, t0, t1),
                                         start=(k == 0), stop=(k == NK - 1))
                for m, (c0, cw) in enumerate(CH):
                    nc.scalar.activation(xi_sb[m][:, t0:t1], pxi[m][:],
                                         AF.Identity, bias=bias_sb[m][:, 0:1])

            a_pass(448, 960, xt_hi)
            a_pass(960, 1152, xt_hi)

            def conv(t0, tw):           # conv outputs for tokens [t0, t0+tw)
                for m, (c0, cw) in enumerate(CH):
                    pc = ps_pool.tile([cw, tw], F32, tag="pc", bufs=3,
                                      name=f"pconv{t0}_{m}")
                    for kc, (k0, kw) in enumerate(CH):
                        for j in range(D_CONV):
                            nc.tensor.matmul(
                                pc[:],
                                wc_sb[kc][:, j * GC + c0:j * GC + c0 + cw],
                                xi_sb[kc][:, t0 + j * BATCH:
                                           t0 + j * BATCH + tw],
                                start=(kc == 0 and j == 0),
                                stop=(kc == 2 and j == D_CONV - 1))
                    nc.scalar.activation(xc3(m, t0, t0 + tw), pc[:], AF.Silu,
                                         bias=bias_sb[m][:, 1:2])

            conv(448, 512)

            pxp = ps_pool.tile([DT_RANK, TOKZ], F32, tag="pc", bufs=3,
                               name="pxp")
            for kc, (k0, kw) in enumerate(CH):
                nc.tensor.matmul(pxp[:], wx_sb[kc][:], xc3(kc, 448, 960),
                                 start=(kc == 0), stop=(kc == 2))
            xp_sb = sc_pool.tile([DT_RANK, TOKZ], F16, tag="xp", name="xp")
            nc.scalar.activation(xp_sb[:], pxp[:], AF.Identity,
                                 bias=bxp_sb[:, 0:1])
            nc.sync.dma_start(out=xp_part[:], in_=xp_sb[:])
            nc.gpsimd.collective_compute(
                "AllReduce", ALU.add, replica_groups=[list(range(8))],
                ins=[xp_part.opt()], outs=[xp_red.opt()])
            xps = sc_pool.tile([DT_RANK, TOKZ], F16, tag="xps", name="xps")
            nc.gpsimd.dma_start(out=xps[:], in_=xp_red[:])

            # ---- while the AR flies: rest of A, conv(0..448), dt-proj, gate
            a_pass(0, 448, xt_lo)
            conv(0, 448)

            for m, (c0, cw) in enumerate(CH):
                pdt = ps_pool.tile([cw, TOKZ], F32, tag="pc", bufs=3,
                                   name=f"pdt{m}")
                nc.tensor.matmul(pdt[:], wdt_sb[:, c0:c0 + cw], xps[:],
                                 start=True, stop=True)
                nc.scalar.activation(usp_all[0:cw, m * TOKZ:(m + 1) * TOKZ],
                                     pdt[:], AF.Exp)
            nc.scalar.activation(dt_all[:], usp_all[:], AF.Ln, bias=1.0)
            for k in range(1, 8):
                nc.scalar.activation(ee2(k), dt_all[:], AF.Exp,
                                     scale=float(k), bias=cb[:, k - 1:k])

            for m, (c0, cw) in enumerate(CH):
                pz = ps_pool.tile([cw, TOKZ], F32, tag="pz", name="pz")
                for k in range(NK):
                    nc.tensor.matmul(pz[:], w_z(k, c0, cw),
                                     xt_hi(k, TOK - TOKZ, TOK),
                                     start=(k == 0), stop=(k == NK - 1))
                nc.scalar.activation(sigz_all[0:cw, m * TOKZ:(m + 1) * TOKZ],
                                     pz[:], AF.Sigmoid,
                                     bias=bias_sb[m][:, 2:3])

            # ---- windowed softmax on merged tiles --------------------------
            # S = 1 + sum_k E_k ; num = sum_k E_k * xc<<k ; E_k from Act.
            W3 = 3 * TOKZ
            S = act_pool.tile([128, W3], BF16, tag="S", name="S")
            num = act_pool.tile([128, W3], BF16, tag="num", name="num")
            nc.vector.tensor_scalar_add(S[:], ee2(1), 1.0)
            nc.vector.tensor_mul(num[:, :].rearrange("p (m t) -> p m t", m=3),
                                 eev(7), xcv(7))
            t1g = tmp_pool.tile([128, W3], BF16, tag="tmpg", bufs=2,
                                name="tg1")
            t2g = tmp_pool.tile([128, W3], BF16, tag="tmpg", bufs=2,
                                name="tg2")
            nc.gpsimd.tensor_mul(t1g[:, :].rearrange("p (m t) -> p m t", m=3),
                                 eev(1), xcv(1))
            nc.gpsimd.tensor_mul(t2g[:, :].rearrange("p (m t) -> p m t", m=3),
                                 eev(2), xcv(2))
            for k in (2, 3, 4, 6, 7):
                nc.vector.tensor_add(S[:], S[:], ee2(k))
            nc.gpsimd.tensor_add(S[:], S[:], ee2(5))
            for k in (6, 5, 4, 3):
                tv = tmp_pool.tile([128, W3], BF16, tag="tmp", name=f"t{k}")
                nc.vector.tensor_mul(
                    tv[:, :].rearrange("p (m t) -> p m t", m=3),
                    eev(k), xcv(k))
                nc.vector.tensor_add(num[:], num[:], tv[:])
            nc.vector.tensor_add(num[:], num[:], t2g[:])
            nc.vector.tensor_add(num[:], num[:], t1g[:])
            nc.vector.tensor_add(num[:, :].rearrange("p (m t) -> p m t", m=3),
                                 num[:, :].rearrange("p (m t) -> p m t", m=3),
                                 xcv(0))

            sf = tmp_pool.tile([128, W3], F32, tag="sf", bufs=1, name="sf")
            nc.scalar.copy(sf[:], S[:])
            sinv = tmp_pool.tile([128, W3], F32, tag="sinv", bufs=1,
                                 name="sinv")
            nc.vector.reciprocal_approx_fast(out=sinv[:], in_=sf[:])
            loc = tmp_pool.tile([128, W3], F16, tag="loc", bufs=1, name="loc")
            nc.vector.tensor_mul(loc[:], num[:], sinv[:])
            ys = tmp_pool.tile([128, W3], F16, tag="ys", bufs=1, name="ys")
            for m, (c0, cw) in enumerate(CH):
                nc.vector.scalar_tensor_tensor(
                    ys[0:cw, m * TOKZ:(m + 1) * TOKZ],
                    xc3(m, 448, 960), bias_sb[m][:, 3:4],
                    loc[0:cw, m * TOKZ:(m + 1) * TOKZ],
                    op0=ALU.mult, op1=ALU.add)
            nc.gpsimd.tensor_mul(ys[:], ys[:], sigz_all[:])
            ysv = ys[:, :].rearrange("p (m two t) -> p m two t", m=3, two=2)
            tr1 = tmp_pool.tile([128, 3 * 256], F16, tag="tmp", name="tr1")
            nc.vector.tensor_add(
                tr1[:, :].rearrange("p (m t) -> p m t", m=3),
                ysv[:, :, 0], ysv[:, :, 1])
            t1v = tr1[:, :].rearrange("p (m two t) -> p m two t", m=3, two=2)
            tr2 = tmp_pool.tile([128, 3 * 128], F16, tag="tmp", name="tr2")
            nc.vector.tensor_add(
                tr2[:, :].rearrange("p (m t) -> p m t", m=3),
                t1v[:, :, 0], t1v[:, :, 1])
            t2v = tr2[:, :].rearrange("p (m two t) -> p m two t", m=3, two=2)
            cext_all = sc_pool.tile([128, 3 * BATCH], F16, tag="cext",
                                    name="cext")
            nc.vector.tensor_add(
                cext_all[:, :].rearrange("p (m t) -> p m t", m=3),
                t2v[:, :, 0], t2v[:, :, 1])

            # hoist the sqrt act-table load ahead of the output AllReduce
            dumm = sc_pool.tile([1, 1], F32, tag="dumm", name="dumm")
            nc.scalar.activation(dumm[:], cb[0:1, 7:8], AF.Sqrt)

            # ---- out partial = cext @ woT (+b_out row), AllReduce ----------
            po = [ps_pool.tile([BATCH, 512], F32,
                               tag=(f"pxi{n}" if n < 3 else "pz"),
                               name=f"po{n}")
                  for n in range(4)]
            for kc in range(4):
                lhs = (cext_all[0:CH[kc][1], kc * BATCH:(kc + 1) * BATCH]
                       if kc < 3 else ones1[:])
                for n in range(4):
                    nc.tensor.matmul(po[n][:], lhs,
                                     wo_sb[kc][:, n * 512:(n + 1) * 512],
                                     start=(kc == 0), stop=(kc == 3))
            outp = sc_pool.tile([BATCH, D_MODEL], F16, tag="outp", name="outp")
            for n in range(4):
                nc.scalar.activation(outp[:, n * 512:(n + 1) * 512],
                                     po[n][:], AF.Copy)
            nc.sync.dma_start(out=op_part[:], in_=outp[:])
            nc.gpsimd.collective_compute(
                "AllReduce", ALU.add, replica_groups=[list(range(8))],
                ins=[op_part.opt()], outs=[op_red.opt()])
            osb = sc_pool.tile([BATCH, D_MODEL], F16, tag="osb", name="osb")
            nc.gpsimd.dma_start(out=osb[:], in_=op_red[:])

            # ---- layernorm over d_model (free dim) -------------------------
            mu = sc_pool.tile([BATCH, 1], F32, tag="mu", name="mu")
            nc.vector.reduce_sum(mu[:], osb[:], axis=mybir.AxisListType.X)
            mus = sc_pool.tile([BATCH, 1], F32, tag="mus", name="mus")
            nc.scalar.mul(mus[:], mu[:], 1.0 / D_MODEL)
            cen = sc_pool.tile([BATCH, D_MODEL], F16, tag="cen", name="cen")
            nc.vector.tensor_scalar_sub(cen[:], osb[:], mus[:])
            sq = sc_pool.tile([BATCH, D_MODEL], F16, tag="outp", name="sq")
            vs = sc_pool.tile([BATCH, 1], F32, tag="vs", name="vs")
            nc.scalar.activation(sq[:], cen[:], AF.Square, accum_out=vs[:])
            std = sc_pool.tile([BATCH, 1], F32, tag="std", name="std")
            nc.scalar.activation(std[:], vs[:], AF.Sqrt,
                                 scale=1.0 / D_MODEL, bias=cb[0:BATCH, 7:8])
            rstd = sc_pool.tile([BATCH, 1], F32, tag="rstd", name="rstd")
            nc.vector.reciprocal(rstd[:], std[:])
            for n in range(4):
                pw = ps_pool.tile([BATCH, 512], F32,
                                  tag=(f"pxi{n}" if n < 3 else "pz"),
                                  name="pw")
                pb = ps_pool.tile([BATCH, 512], F32, tag="pc", bufs=3,
                                  name="pb")
                nc.tensor.matmul(pw[:], ones1[:],
                                 lnw_sb[:, n * 512:(n + 1) * 512],
                                 start=True, stop=True)
                nc.tensor.matmul(pb[:], ones1[:],
                                 lnb_sb[:, n * 512:(n + 1) * 512],
                                 start=True, stop=True)
                fin = sc_pool.tile([BATCH, 512], F32, tag="fin", bufs=2,
                                   name=f"fin{n}")
                nc.vector.scalar_tensor_tensor(
                    fin[:], cen[:, n * 512:(n + 1) * 512], rstd[:], pw[:],
                    op0=ALU.mult, op1=ALU.mult)
                nc.vector.tensor_add(fin[:], fin[:], pb[:])
                nc.sync.dma_start(out=out[:, n * 512:(n + 1) * 512],
                                  in_=fin[:])

    nc.compile()
    return nc


def _host_prep(inputs):
    f = lambda k: np.ascontiguousarray(np.asarray(inputs[k], dtype=np.float32))
    x, W_in, b_in = f("x"), f("W_in"), f("b_in")
    W_gate, b_gate = f("W_gate"), f("b_gate")
    W_conv, b_conv = f("W_conv"), f("b_conv")
    W_xproj, b_xproj = f("W_xproj"), f("b_xproj")
    W_dt, Dparam = f("W_dt"), f("Dparam")
    W_out, b_out = f("W_out"), f("b_out")
    ln_w, ln_b = f("ln_w"), f("ln_b")

    xT = np.ascontiguousarray(
        x[SEQ - NPOS:].reshape(TOK, D_MODEL).T).astype(np.float16)
    lnwb = np.ascontiguousarray(np.stack([ln_w, ln_b])).astype(np.float16)

    in_maps = []
    for g in range(8):
        if g < GROUPS:
            ch = slice(GC * g, GC * (g + 1))
            wigm = np.concatenate([W_in[ch].T, W_gate[ch].T], axis=1)
            wcm = np.ascontiguousarray(
                W_conv[ch].transpose(1, 2, 0).reshape(GC, D_CONV * GC))
            wom = np.zeros((GC + 1, D_MODEL), np.float32)
            wom[:GC] = W_out[:, ch].T / float(WIN)
            if g == 0:
                wom[GC] = b_out
            wxm = np.ascontiguousarray(W_xproj[:DT_RANK, ch].T)
            wdtm = np.ascontiguousarray(W_dt[ch].T)
            biasm = np.ascontiguousarray(
                np.stack([b_in[ch], b_conv[ch], b_gate[ch], Dparam[ch]], 1))
            bxpm = (b_xproj[:DT_RANK] if g == 0
                    else np.zeros(DT_RANK, np.float32)).reshape(DT_RANK, 1)
            bxpm = np.ascontiguousarray(bxpm)
        else:
            wigm = np.zeros((D_MODEL, 2 * GC), np.float32)
            wcm = np.zeros((GC, D_CONV * GC), np.float32)
            wom = np.zeros((GC + 1, D_MODEL), np.float32)
            wxm = np.zeros((GC, DT_RANK), np.float32)
            wdtm = np.zeros((DT_RANK, GC), np.float32)
            biasm = np.zeros((GC, 4), np.float32)
            bxpm = np.zeros((DT_RANK, 1), np.float32)
        in_maps.append({
            "xT": xT,
            "wig": np.ascontiguousarray(wigm).astype(np.float16),
            "wc": wcm.astype(np.float16),
            "wo": wom.astype(np.float16),
            "wx": wxm.astype(np.float16),
            "wdt": wdtm.astype(np.float16),
            "biasv": biasm, "bxp": bxpm, "lnwb": lnwb,
        })
    return in_maps


def kernel(**inputs):
    if "nc" not in _cache:
        _cache["nc"] = _build()
    in_maps = _host_prep(inputs)
    res = run_bass_kernel_spmd(_cache["nc"], in_maps, list(range(8)))
    return res.results[0]["out"]


# revision 18
# speedup vs baseline: 1.4533x; 1.0090x over previous
"""Trainium2 Bass kernel for nn_MemoryEfficientS6Compressor.

Key insight: the reference returns LN(W_out @ mean(ys[-8:]) + b_out) where
ys[-8:] are the last 8 positions of the LAST chunk (chunk-local t=24..31).
Chunks are independent, so only chunk 3 matters, and within it only:
  - xi (W_in proj) for chunk-local positions 14..31  (18 positions)
  - conv+silu (xc) for positions 17..31              (15 positions)
  - dt / gate / window-softmax for positions 24..31  (8 positions)

Sharding: 7 conv groups (351 channels) -> cores 0..6; core 7 runs the same
SPMD program on zeroed weights. Cross-core sums (x_proj partials, W_out
partials) via AllReduce.

Schedule (all fp16 matmuls; bf16 only for exp-range tiles):
  1. DMA the xp-critical slices first (x cols 448.., W_in block) so the
     x_proj AllReduce launches ~35us in instead of ~90us.
  2. A-passes over tokens 448..960, 960..1152 -> conv(448..960) -> xp ->
     AllReduce(fp16).  While it flies: A-pass 0..448, conv(0..448),
     dt-proj, gate.
  3. Windowed softmax on merged [128, 3*512] tiles (3 channel-chunks per
     op) split across Act (table exps) / DVE / Pool.
  4. out partial -> fp16 AllReduce (Shared output) -> layernorm.
"""

import os

import numpy as np

import concourse.bass as bass
import concourse.mybir as mybir
from concourse import bacc
from concourse.bass_utils import run_bass_kernel_spmd
from concourse.tile import TileContext

F32 = mybir.dt.float32
F16 = mybir.dt.float16
BF16 = mybir.dt.bfloat16
AF = mybir.ActivationFunctionType
ALU = mybir.AluOpType

SEQ, BATCH, D_MODEL = 128, 64, 2048
D_INNER, GROUPS, D_CONV = 2457, 7, 4
DT_RANK, WIN = 32, 8
GC = D_INNER // GROUPS          # 351 channels per group
NPOS = 18                        # xi positions (chunk-local 14..31)
NCONV = 15                       # conv output positions (17..31)
TOK = NPOS * BATCH               # 1152
TOKC = NCONV * BATCH             # 960
TOKZ = 8 * BATCH                 # 512
CH = [(0, 128), (128, 128), (256, 95)]
NK = D_MODEL // 128              # 16 k-chunks over d_model
HI = 448                         # token split: [448..1152] is xp-critical

_cache = {}


def _build():
    nc = bacc.Bacc("TRN2", target_bir_lowering=False, debug=False,
                   num_devices=8)

    xT = nc.dram_tensor("xT", [D_MODEL, TOK], F16, kind="ExternalInput").ap()
    wig = nc.dram_tensor("wig", [D_MODEL, 2 * GC], F16, kind="ExternalInput").ap()
    wc = nc.dram_tensor("wc", [GC, D_CONV * GC], F16, kind="ExternalInput").ap()
    wo = nc.dram_tensor("wo", [GC + 1, D_MODEL + 1], F16, kind="ExternalInput").ap()
    wx = nc.dram_tensor("wx", [GC, DT_RANK], F16, kind="ExternalInput").ap()
    wdt = nc.dram_tensor("wdt", [DT_RANK, GC], F16, kind="ExternalInput").ap()
    biasv = nc.dram_tensor("biasv", [GC, 4], F32, kind="ExternalInput").ap()
    bxp = nc.dram_tensor("bxp", [DT_RANK, 1], F32, kind="ExternalInput").ap()
    lnwb = nc.dram_tensor("lnwb", [2, D_MODEL], F16, kind="ExternalInput").ap()
    out = nc.dram_tensor("out", [BATCH, D_MODEL], F32, kind="ExternalOutput").ap()

    xp_part = nc.dram_tensor("xp_part", [DT_RANK, TOKZ], F16,
                             kind="Internal").ap()
    xp_red = nc.dram_tensor("xp_red", [DT_RANK, TOKZ], F16,
                            kind="Internal", addr_space="Shared").ap()
    op_part = nc.dram_tensor("op_part", [BATCH, D_MODEL + 1], F16,
                             kind="Internal").ap()
    op_red = nc.dram_tensor("op_red", [BATCH, D_MODEL + 1], F16,
                            kind="Internal", addr_space="Shared").ap()

    with TileContext(nc) as tc:
        with (
            tc.tile_pool(name="xt", bufs=1) as xt_pool,
            tc.tile_pool(name="wig", bufs=1) as wig_pool,
            tc.tile_pool(name="wgt", bufs=1) as wgt_pool,
            tc.tile_pool(name="wo", bufs=1) as wo_pool,
            tc.tile_pool(name="act", bufs=1) as act_pool,
            tc.tile_pool(name="ek", bufs=1) as ek_pool,
            tc.tile_pool(name="tmp", bufs=3) as tmp_pool,
            tc.tile_pool(name="sc", bufs=1) as sc_pool,
            tc.tile_pool(name="ps", bufs=1, space="PSUM") as ps_pool,
        ):
            # ---- DMA: xp-critical first (x hi-cols + W_in), paired k ------
            # host packs wig = [W_in.T | W_gate.T] (each [2048, 351])
            xthi_sb, wxi_sb = [], []
            for p in range(NK // 2):
                th = xt_pool.tile([128, 2, TOK - HI], F16, tag=f"xth{p}",
                                  name=f"xth{p}")
                nc.sync.dma_start(
                    out=th[:, :, :],
                    in_=xT[256 * p:256 * (p + 1), HI:].rearrange(
                        "(two p) t -> p two t", two=2))
                xthi_sb.append(th)
                tw = wig_pool.tile([128, 2, GC], F16, tag=f"wxi{p}",
                                   name=f"wxi{p}")
                nc.sync.dma_start(
                    out=tw[:, :, :],
                    in_=wig[256 * p:256 * (p + 1), 0:GC].rearrange(
                        "(two p) t -> p two t", two=2))
                wxi_sb.append(tw)
            # deferred: x lo-cols + W_gate
            xtlo_sb, wz_sb = [], []
            for p in range(NK // 2):
                tl = xt_pool.tile([128, 2, HI], F16, tag=f"xtl{p}",
                                  name=f"xtl{p}")
                nc.sync.dma_start(
                    out=tl[:, :, :],
                    in_=xT[256 * p:256 * (p + 1), 0:HI].rearrange(
                        "(two p) t -> p two t", two=2))
                xtlo_sb.append(tl)
                tz = wig_pool.tile([128, 2, GC], F16, tag=f"wz{p}",
                                   name=f"wz{p}")
                nc.sync.dma_start(
                    out=tz[:, :, :],
                    in_=wig[256 * p:256 * (p + 1), GC:].rearrange(
                        "(two p) t -> p two t", two=2))
                wz_sb.append(tz)

            def xt_hi(k, t0, t1):       # tokens in [HI, 1152)
                return xthi_sb[k // 2][:, k % 2, t0 - HI:t1 - HI]

            def xt_lo(k, t0, t1):       # tokens in [0, HI)
                return xtlo_sb[k // 2][:, k % 2, t0:t1]

            def w_xi(k, c0, cw):
                return wxi_sb[k // 2][:, k % 2, c0:c0 + cw]

            def w_z(k, c0, cw):
                return wz_sb[k // 2][:, k % 2, c0:c0 + cw]

            # small tiles on the scalar queue
            bias_sb = []
            for m, (c0, cw) in enumerate(CH):
                b = sc_pool.tile([cw, 4], F32, tag=f"bias{m}", name=f"bias{m}")
                nc.scalar.dma_start(out=b[:], in_=biasv[c0:c0 + cw, :])
                bias_sb.append(b)
            wx_sb = []
            for m, (c0, cw) in enumerate(CH):
                t = sc_pool.tile([cw, DT_RANK], F16, tag=f"wx{m}", name=f"wx{m}")
                nc.scalar.dma_start(out=t[:], in_=wx[c0:c0 + cw, :])
                wx_sb.append(t)
            wdt_sb = sc_pool.tile([DT_RANK, GC], F16, tag="wdt", name="wdt")
            nc.scalar.dma_start(out=wdt_sb[:], in_=wdt[:, :])
            bxp_sb = sc_pool.tile([DT_RANK, 1], F32, tag="bxp", name="bxp")
            nc.scalar.dma_start(out=bxp_sb[:], in_=bxp[:, :])
            lnw_sb = sc_pool.tile([1, D_MODEL], F16, tag="lnw", name="lnw")
            nc.scalar.dma_start(out=lnw_sb[:], in_=lnwb[0:1, :])
            lnb_sb = sc_pool.tile([1, D_MODEL], F16, tag="lnb", name="lnb")
            nc.scalar.dma_start(out=lnb_sb[:], in_=lnwb[1:2, :])

            ones1 = sc_pool.tile([1, BATCH], F16, tag="ones1", name="ones1")
            nc.vector.memset(ones1[:], 1.0)
            cb = sc_pool.tile([128, 1], F32, tag="cb", name="cb")
            nc.vector.memset(cb[:, 0:1], 1e-5)

            # conv + out weights on the gpsimd queue
            wc_sb = []
            for m, (c0, cw) in enumerate(CH):
                t = wgt_pool.tile([cw, D_CONV * GC], F16, tag=f"wc{m}",
                                  name=f"wc{m}")
                nc.gpsimd.dma_start(out=t[:], in_=wc[c0:c0 + cw, :])
                wc_sb.append(t)
            wo_rows = [(0, 128), (128, 128), (256, 95), (351, 1)]
            wo_sb = []
            for r, (r0, rw) in enumerate(wo_rows):
                t = wo_pool.tile([rw, D_MODEL + 1], F16, tag=f"wo{r}", name=f"wo{r}")
                nc.gpsimd.dma_start(out=t[:], in_=wo[r0:r0 + rw, :])
                wo_sb.append(t)

            # merged activation tiles: [128, 3*N] spanning channel-chunks
            xi_sb = [act_pool.tile([cw, TOK], F16, tag=f"xi{m}", name=f"xi{m}")
                     for m, (c0, cw) in enumerate(CH)]
            xc_all = act_pool.tile([128, 3 * TOKC], F16, tag="xc", name="xc")
            sigz_all = act_pool.tile([128, 3 * TOKZ], F16, tag="sigz",
                                     name="sigz")
            usp_all = act_pool.tile([128, 3 * TOKZ], F16, tag="usp", name="usp")
            nc.vector.memset(usp_all[64:128, 2 * TOKZ:3 * TOKZ], 0.0)
            nc.vector.memset(sigz_all[64:128, 2 * TOKZ:3 * TOKZ], 0.0)
            nc.vector.memset(xc_all[64:128, 2 * TOKC:3 * TOKC], 0.0)
            dt_all = act_pool.tile([128, 3 * TOKZ], F32, tag="dt", name="dt")
            EE = ek_pool.tile([128, 7 * 3 * TOKZ], BF16, tag="EE", name="EE")

            def xc3(m, t0, t1):
                return xc_all[0:CH[m][1], m * TOKC + t0:m * TOKC + t1]

            def xcv(k):     # [128, 3, 512] window view
                v = xc_all[:, :].rearrange("p (m t) -> p m t", m=3)
                return v[:, :, k * BATCH:k * BATCH + TOKZ]

            def eev(k):     # [128, 3, 512] view of plane k
                v = EE[:, (k - 1) * 3 * TOKZ:k * 3 * TOKZ]
                return v.rearrange("p (m t) -> p m t", m=3)

            def ee2(k):     # [128, 1536] view of plane k
                return EE[:, (k - 1) * 3 * TOKZ:k * 3 * TOKZ]

            # ---- A-pass over hi tokens (448..960 + 960..1152 share one
            # ---- ldweights per (k, m)), then conv(448..960), xp, AR --------
            pxi = [ps_pool.tile([cw, 512], F32, tag=f"pxi{m}",
                                name=f"pxi{m}_hi")
                   for m, (c0, cw) in enumerate(CH)]
            pxj = [ps_pool.tile([cw, 192], F32, tag="pc", bufs=3,
                                name=f"pxj{m}_hi")
                   for m, (c0, cw) in enumerate(CH)]
            for k in range(NK):
                for m, (c0, cw) in enumerate(CH):
                    nc.tensor.matmul(pxi[m][:], w_xi(k, c0, cw),
                                     xt_hi(k, 448, 960),
                                     start=(k == 0), stop=(k == NK - 1))
                    nc.tensor.matmul(pxj[m][:], w_xi(k, c0, cw),
                                     xt_hi(k, 960, 1152),
                                     start=(k == 0), stop=(k == NK - 1),
                                     skip_group_check=True)
            for m, (c0, cw) in enumerate(CH):
                nc.scalar.activation(xi_sb[m][:, 448:960], pxi[m][:],
                                     AF.Identity, bias=bias_sb[m][:, 0:1])
                nc.scalar.activation(xi_sb[m][:, 960:1152], pxj[m][:],
                                     AF.Identity, bias=bias_sb[m][:, 0:1])

            def a_pass(t0, t1, xt_f):
                pxa = [ps_pool.tile([cw, t1 - t0], F32, tag=f"pxi{m}",
                                    name=f"pxi{m}_{t0}")
                       for m, (c0, cw) in enumerate(CH)]
                for k in range(NK):
                    for m, (c0, cw) in enumerate(CH):
                        nc.tensor.matmul(pxa[m][:], w_xi(k, c0, cw),
                                         xt_f(k, t0, t1),
                                         start=(k == 0), stop=(k == NK - 1))
                for m, (c0, cw) in enumerate(CH):
                    nc.scalar.activation(xi_sb[m][:, t0:t1], pxa[m][:],
                                         AF.Identity, bias=bias_sb[m][:, 0:1])

            def conv(t0, tw):           # conv outputs for tokens [t0, t0+tw)
                for m, (c0, cw) in enumerate(CH):
                    pc = ps_pool.tile([cw, tw], F32, tag="pc", bufs=3,
                                      name=f"pconv{t0}_{m}")
                    for kc, (k0, kw) in enumerate(CH):
                        for j in range(D_CONV):
                            nc.tensor.matmul(
                                pc[:],
                                wc_sb[kc][:, j * GC + c0:j * GC + c0 + cw],
                                xi_sb[kc][:, t0 + j * BATCH:
                                           t0 + j * BATCH + tw],
                                start=(kc == 0 and j == 0),
                                stop=(kc == 2 and j == D_CONV - 1))
                    nc.scalar.activation(xc3(m, t0, t0 + tw), pc[:], AF.Silu,
                                         bias=bias_sb[m][:, 1:2])

            conv(448, 512)

            pxp = ps_pool.tile([DT_RANK, TOKZ], F32, tag="pc", bufs=3,
                               name="pxp")
            for kc, (k0, kw) in enumerate(CH):
                nc.tensor.matmul(pxp[:], wx_sb[kc][:], xc3(kc, 448, 960),
                                 start=(kc == 0), stop=(kc == 2))
            xp_sb = sc_pool.tile([DT_RANK, TOKZ], F16, tag="xp", name="xp")
            nc.scalar.activation(xp_sb[:], pxp[:], AF.Identity,
                                 bias=bxp_sb[:, 0:1])
            nc.sync.dma_start(out=xp_part[:], in_=xp_sb[:])
            nc.gpsimd.collective_compute(
                "AllReduce", ALU.add, replica_groups=[list(range(8))],
                ins=[xp_part.opt()], outs=[xp_red.opt()])
            xps = sc_pool.tile([DT_RANK, TOKZ], F16, tag="xps", name="xps")
            nc.gpsimd.dma_start(out=xps[:], in_=xp_red[:])

            # ---- while the AR flies: rest of A, conv(0..448), dt-proj, gate
            a_pass(0, 448, xt_lo)
            conv(0, 448)

            for m, (c0, cw) in enumerate(CH):
                pdt = ps_pool.tile([cw, TOKZ], F32, tag="pc", bufs=3,
                                   name=f"pdt{m}")
                nc.tensor.matmul(pdt[:], wdt_sb[:, c0:c0 + cw], xps[:],
                                 start=True, stop=True)
                nc.scalar.activation(usp_all[0:cw, m * TOKZ:(m + 1) * TOKZ],
                                     pdt[:], AF.Exp)
            # E_k = (1+e^u)^k via one Identity + squares (all same act table)
            nc.scalar.activation(ee2(1), usp_all[:], AF.Identity, bias=1.0)
            nc.scalar.activation(ee2(2), ee2(1), AF.Square)
            nc.scalar.activation(ee2(4), ee2(2), AF.Square)
            nc.vector.tensor_mul(ee2(3), ee2(1), ee2(2))
            nc.scalar.activation(ee2(6), ee2(3), AF.Square)
            nc.vector.tensor_mul(ee2(5), ee2(1), ee2(4))
            nc.vector.tensor_mul(ee2(7), ee2(3), ee2(4))

            for m, (c0, cw) in enumerate(CH):
                pz = ps_pool.tile([cw, TOKZ], F32, tag="pc", bufs=3, name="pz")
                for k in range(NK):
                    nc.tensor.matmul(pz[:], w_z(k, c0, cw),
                                     xt_hi(k, TOK - TOKZ, TOK),
                                     start=(k == 0), stop=(k == NK - 1))
                nc.scalar.activation(sigz_all[0:cw, m * TOKZ:(m + 1) * TOKZ],
                                     pz[:], AF.Sigmoid,
                                     bias=bias_sb[m][:, 2:3])

            # ---- windowed softmax on merged tiles --------------------------
            # S = 1 + sum_k E_k ; num = sum_k E_k * xc<<k ; E_k from Act.
            W3 = 3 * TOKZ
            S = act_pool.tile([128, W3], BF16, tag="S", name="S")
            num = act_pool.tile([128, W3], BF16, tag="num", name="num")
            r3 = lambda ap: ap.rearrange("p (m t) -> p m t", m=3)
            # S = 1 + sum E_k (gp helps with two adds, off critical path)
            nc.vector.tensor_scalar_add(S[:], ee2(1), 1.0)
            for k in (2, 3, 4, 6):
                nc.vector.tensor_add(S[:], S[:], ee2(k))
            nc.gpsimd.tensor_add(S[:], S[:], ee2(5))
            nc.gpsimd.tensor_add(S[:], S[:], ee2(7))
            # pairwise product tree for num
            tvs = {}
            for k in range(1, 8):
                tv = tmp_pool.tile([128, W3], BF16, tag="tmp", bufs=8,
                                   name=f"t{k}")
                nc.vector.tensor_mul(r3(tv[:]), eev(k), xcv(k))
                tvs[k] = tv
            a10 = tvs[1]
            nc.vector.tensor_add(r3(a10[:]), r3(tvs[1][:]), xcv(0))
            a32 = tvs[3]
            nc.vector.tensor_add(a32[:], tvs[3][:], tvs[2][:])
            a54 = tvs[5]
            nc.vector.tensor_add(a54[:], tvs[5][:], tvs[4][:])
            a76 = tvs[7]
            nc.vector.tensor_add(a76[:], tvs[7][:], tvs[6][:])
            nc.vector.tensor_add(a10[:], a10[:], a32[:])
            nc.vector.tensor_add(a54[:], a54[:], a76[:])
            nc.vector.tensor_add(num[:], a10[:], a54[:])

            sf = tmp_pool.tile([128, W3], F32, tag="sf", bufs=1, name="sf")
            nc.scalar.copy(sf[:], S[:])
            sinv = tmp_pool.tile([128, W3], F32, tag="sinv", bufs=1,
                                 name="sinv")
            nc.vector.reciprocal_approx_fast(out=sinv[:], in_=sf[:])
            loc = tmp_pool.tile([128, W3], F16, tag="loc", bufs=1, name="loc")
            nc.vector.tensor_mul(loc[:], num[:], sinv[:])
            ys = tmp_pool.tile([128, W3], F16, tag="ys", bufs=1, name="ys")
            for m, (c0, cw) in enumerate(CH):
                nc.vector.scalar_tensor_tensor(
                    ys[0:cw, m * TOKZ:(m + 1) * TOKZ],
                    xc3(m, 448, 960), bias_sb[m][:, 3:4],
                    loc[0:cw, m * TOKZ:(m + 1) * TOKZ],
                    op0=ALU.mult, op1=ALU.add)
            nc.gpsimd.tensor_mul(ys[:], ys[:], sigz_all[:])
            ysv = ys[:, :].rearrange("p (m two t) -> p m two t", m=3, two=2)
            tr1 = tmp_pool.tile([128, 3 * 256], F16, tag="tr1", bufs=1, name="tr1")
            nc.vector.tensor_add(
                tr1[:, :].rearrange("p (m t) -> p m t", m=3),
                ysv[:, :, 0], ysv[:, :, 1])
            t1v = tr1[:, :].rearrange("p (m two t) -> p m two t", m=3, two=2)
            tr2 = tmp_pool.tile([128, 3 * 128], F16, tag="tr2", bufs=1, name="tr2")
            nc.vector.tensor_add(
                tr2[:, :].rearrange("p (m t) -> p m t", m=3),
                t1v[:, :, 0], t1v[:, :, 1])
            t2v = tr2[:, :].rearrange("p (m two t) -> p m two t", m=3, two=2)
            cext_all = sc_pool.tile([128, 3 * BATCH], F16, tag="cext",
                                    name="cext")
            nc.vector.tensor_add(
                cext_all[:, :].rearrange("p (m t) -> p m t", m=3),
                t2v[:, :, 0], t2v[:, :, 1])

            # hoist the sqrt act-table load ahead of the output AllReduce
            # (write into osb so the op isn't dead-code eliminated; the AR
            # result DMA fully overwrites it afterwards)

            # ---- out partial = cext @ woT (+b_out row), AllReduce ----------
            po = [ps_pool.tile([BATCH, 512], F32,
                               tag=(f"pxi{n}" if n < 3 else "po3"),
                               name=f"po{n}")
                  for n in range(4)]
            pomu = ps_pool.tile([BATCH, 1], F32, tag="pc", bufs=3,
                                name="pomu")
            for kc in range(4):
                lhs = (cext_all[0:CH[kc][1], kc * BATCH:(kc + 1) * BATCH]
                       if kc < 3 else ones1[:])
                for n in range(4):
                    nc.tensor.matmul(po[n][:], lhs,
                                     wo_sb[kc][:, n * 512:(n + 1) * 512],
                                     start=(kc == 0), stop=(kc == 3))
                nc.tensor.matmul(pomu[:], lhs,
                                 wo_sb[kc][:, D_MODEL:D_MODEL + 1],
                                 start=(kc == 0), stop=(kc == 3),
                                 skip_group_check=True)
            outp = sc_pool.tile([BATCH, D_MODEL + 1], F16, tag="outp",
                                name="outp")
            osb = sc_pool.tile([BATCH, D_MODEL + 1], F16, tag="osb",
                               name="osb")
            for n in range(4):
                nc.scalar.activation(outp[:, n * 512:(n + 1) * 512],
                                     po[n][:], AF.Copy)
            nc.scalar.activation(outp[:, D_MODEL:D_MODEL + 1], pomu[:],
                                 AF.Copy)
            nc.sync.dma_start(out=op_part[:], in_=outp[:])
            nc.gpsimd.collective_compute(
                "AllReduce", ALU.add, replica_groups=[list(range(8))],
                ins=[op_part.opt()], outs=[op_red.opt()])
            nc.scalar.activation(osb[0:1, 0:1], cb[0:1, 0:1], AF.Sqrt)
            nc.gpsimd.dma_start(out=osb[:], in_=op_red[:])

            # ---- layernorm over d_model (free dim) -------------------------
            mus = sc_pool.tile([BATCH, 1], F32, tag="mus", name="mus")
            nc.scalar.mul(mus[:], osb[:, D_MODEL:D_MODEL + 1], 1.0 / D_MODEL)
            cen = sc_pool.tile([BATCH, D_MODEL], F16, tag="cen", name="cen")
            nc.vector.tensor_scalar_sub(cen[:], osb[:, 0:D_MODEL], mus[:])
            sq = sc_pool.tile([BATCH, D_MODEL], F16, tag="outp", name="sq")
            vs = sc_pool.tile([BATCH, 1], F32, tag="vs", name="vs")
            nc.scalar.activation(sq[:], cen[:], AF.Square, accum_out=vs[:])
            std = sc_pool.tile([BATCH, 1], F32, tag="std", name="std")
            nc.scalar.activation(std[:], vs[:], AF.Sqrt,
                                 scale=1.0 / D_MODEL, bias=cb[0:BATCH, 0:1])
            rstd = sc_pool.tile([BATCH, 1], F32, tag="rstd", name="rstd")
            nc.vector.reciprocal(rstd[:], std[:])
            for n in range(4):
                pw = ps_pool.tile([BATCH, 512], F32,
                                  tag=(f"pxi{n}" if n < 3 else "po3"),
                                  name="pw")
                pb = ps_pool.tile([BATCH, 512], F32, tag="pc", bufs=3,
                                  name="pb")
                nc.tensor.matmul(pw[:], ones1[:],
                                 lnw_sb[:, n * 512:(n + 1) * 512],
                                 start=True, stop=True)
                nc.tensor.matmul(pb[:], ones1[:],
                                 lnb_sb[:, n * 512:(n + 1) * 512],
                                 start=True, stop=True)
                fin = sc_pool.tile([BATCH, 512], F32, tag="fin", bufs=2,
                                   name=f"fin{n}")
                nc.vector.scalar_tensor_tensor(
                    fin[:], cen[:, n * 512:(n + 1) * 512], rstd[:], pw[:],
                    op0=ALU.mult, op1=ALU.mult)
                nc.vector.tensor_add(fin[:], fin[:], pb[:])
                nc.sync.dma_start(out=out[:, n * 512:(n + 1) * 512],
                                  in_=fin[:])

    nc.compile()
    return nc


def _host_prep(inputs):
    f = lambda k: np.ascontiguousarray(np.asarray(inputs[k], dtype=np.float32))
    x, W_in, b_in = f("x"), f("W_in"), f("b_in")
    W_gate, b_gate = f("W_gate"), f("b_gate")
    W_conv, b_conv = f("W_conv"), f("b_conv")
    W_xproj, b_xproj = f("W_xproj"), f("b_xproj")
    W_dt, Dparam = f("W_dt"), f("Dparam")
    W_out, b_out = f("W_out"), f("b_out")
    ln_w, ln_b = f("ln_w"), f("ln_b")

    xT = np.ascontiguousarray(
        x[SEQ - NPOS:].reshape(TOK, D_MODEL).T).astype(np.float16)
    lnwb = np.ascontiguousarray(np.stack([ln_w, ln_b])).astype(np.float16)

    in_maps = []
    for g in range(8):
        if g < GROUPS:
            ch = slice(GC * g, GC * (g + 1))
            wigm = np.concatenate([W_in[ch].T, W_gate[ch].T], axis=1)
            wcm = np.ascontiguousarray(
                W_conv[ch].transpose(1, 2, 0).reshape(GC, D_CONV * GC))
            wom = np.zeros((GC + 1, D_MODEL + 1), np.float32)
            wom[:GC, :D_MODEL] = W_out[:, ch].T / float(WIN)
            if g == 0:
                wom[GC, :D_MODEL] = b_out
            wom[:, D_MODEL] = wom[:, :D_MODEL].sum(axis=1)
            wxm = np.ascontiguousarray(W_xproj[:DT_RANK, ch].T)
            wdtm = np.ascontiguousarray(W_dt[ch].T)
            biasm = np.ascontiguousarray(
                np.stack([b_in[ch], b_conv[ch], b_gate[ch], Dparam[ch]], 1))
            bxpm = (b_xproj[:DT_RANK] if g == 0
                    else np.zeros(DT_RANK, np.float32)).reshape(DT_RANK, 1)
            bxpm = np.ascontiguousarray(bxpm)
        else:
            wigm = np.zeros((D_MODEL, 2 * GC), np.float32)
            wcm = np.zeros((GC, D_CONV * GC), np.float32)
            wom = np.zeros((GC + 1, D_MODEL + 1), np.float32)
            wxm = np.zeros((GC, DT_RANK), np.float32)
            wdtm = np.zeros((DT_RANK, GC), np.float32)
            biasm = np.zeros((GC, 4), np.float32)
            bxpm = np.zeros((DT_RANK, 1), np.float32)
        in_maps.append({
            "xT": xT,
            "wig": np.ascontiguousarray(wigm).astype(np.float16),
            "wc": wcm.astype(np.float16),
            "wo": wom.astype(np.float16),
            "wx": wxm.astype(np.float16),
            "wdt": wdtm.astype(np.float16),
            "biasv": biasm, "bxp": bxpm, "lnwb": lnwb,
        })
    return in_maps


def kernel(**inputs):
    if "nc" not in _cache:
        _cache["nc"] = _build()
    in_maps = _host_prep(inputs)
    res = run_bass_kernel_spmd(_cache["nc"], in_maps, list(range(8)))
    return res.results[0]["out"]
